# revision 17
# baseline (speedup 1.0000x reference)
"""Trainium2 Bass kernel for nn_DifferentiableSelectCopy (NTM read/write head).

Computes, for memory (B=64, M=16384, D=64) and controller_state (B=64, C=256):
  w_r, w_w = addressing heads (content + gate + circular shift + sharpening)
  read_content = sum_m w_r[m] * memory[m, :]
  new_memory   = memory * (1 - w_w (x) erase) + w_w (x) add

Sharding: data-parallel over batch, 8 batches per NeuronCore.

Per-core structure:
  Pass 1 (stats): stream memory row-tiles, PE-transpose to D-on-partitions,
    PE matmuls against keys / ones to get rawdot (per head) and row sq-norms.
  Weight math: per (batch, head) on (128, 128) M-vector tiles.
  Pass 2: re-stream memory, fused DVE scalar_tensor_tensor ops produce
    new_memory; PE matmuls accumulate read_content.
"""
import numpy as np

import concourse.bass as bass
import concourse.tile as tile
import concourse.mybir as mybir
from concourse import bacc
from concourse.bass_utils import run_bass_kernel_spmd

f32 = mybir.dt.float32
f32r = mybir.dt.float32r
OP = mybir.AluOpType
AF = mybir.ActivationFunctionType

B, M, D, C, S = 64, 16384, 64, 256, 3
NCORES = 8
BL = B // NCORES            # 8 local batches per core
EPS = 1e-8

USE_F32R = True             # relaxed-precision PE matmuls for stats/readc
DEBUG = False               # extra DRAM outputs of intermediates

# pass-1 tiling: per (b, t): 4096 rows as (128p, 32c, 64d), row = t*4096 + c*128 + p
T1 = M // 4096              # 4
# pass-2 tiling: per tt: 512 rows/batch as (128p, 4j, 64d), row = tt*512 + jj*128 + p
T2 = M // 512               # 32


def _consts():
    ident = np.eye(128, dtype=np.float32)
    ones_row = np.ones((1, 128), dtype=np.float32)
    ones_col = np.ones((128, 1), dtype=np.float32)
    onesblk = np.zeros((128, 2), dtype=np.float32)
    onesblk[0:64, 0] = 1.0
    onesblk[64:128, 1] = 1.0
    nextmat = np.zeros((128, 128), dtype=np.float32)
    nextmat[(np.arange(128) + 1) % 128, np.arange(128)] = 1.0   # out[m'] = x[m'+1]
    prevmat = np.zeros((128, 128), dtype=np.float32)
    prevmat[(np.arange(128) - 1) % 128, np.arange(128)] = 1.0   # out[m'] = x[m'-1]
    return ident, ones_row, ones_col, onesblk, nextmat, prevmat


def build_program():
    nc = bacc.Bacc("TRN2", target_bir_lowering=False, debug=False, num_devices=NCORES)
    mmdt = f32r if USE_F32R else f32

    def cast(ap):
        return ap.bitcast(mmdt)

    # ---- DRAM I/O ----
    mem = nc.dram_tensor("mem", (BL, M, D), f32, kind="ExternalInput").ap()
    ctrlT = nc.dram_tensor("ctrlT", (C, BL), f32, kind="ExternalInput").ap()
    wc0 = nc.dram_tensor("wc0", (C, 128), f32, kind="ExternalInput").ap()  # [Wk_r|Wk_w]
    wc1 = nc.dram_tensor("wc1", (C, 128), f32, kind="ExternalInput").ap()  # [We|Wa]
    wc2 = nc.dram_tensor("wc2", (C, 12), f32, kind="ExternalInput").ap()
    b2d = nc.dram_tensor("b2", (12, 1), f32, kind="ExternalInput").ap()
    bebad = nc.dram_tensor("beba", (128, 1), f32, kind="ExternalInput").ap()
    ivd = nc.dram_tensor("iv", (1, 2), f32, kind="ExternalInput").ap()

    newmem = nc.dram_tensor("newmem", (BL, M, D), f32, kind="ExternalOutput").ap()
    readc_d = nc.dram_tensor("readc", (BL, D), f32, kind="ExternalOutput").ap()
    dbg = {}
    if DEBUG:
        for nm, shp in [("d_rdr", (128, BL, 128)), ("d_rdw", (128, BL, 128)),
                        ("d_nrm", (128, BL, 128)), ("d_wrT", (128, BL, 128)),
                        ("d_wwT", (128, BL, 128))]:
            dbg[nm] = nc.dram_tensor(nm, shp, f32, kind="ExternalOutput").ap()

    c_ident, c_ones_row, c_ones_col, c_onesblk, c_next, c_prev = _consts()
    identd = nc.inline_tensor(c_ident, "c_ident").ap()
    onesrd = nc.inline_tensor(c_ones_row, "c_onesr").ap()
    onescd = nc.inline_tensor(c_ones_col, "c_onesc").ap()
    onesbd = nc.inline_tensor(c_onesblk, "c_onesb").ap()
    nextd = nc.inline_tensor(c_next, "c_next").ap()
    prevd = nc.inline_tensor(c_prev, "c_prev").ap()

    with tile.TileContext(nc) as tc:
        with (
            tc.tile_pool(name="pers", bufs=1) as pers,  # persistent tensors
            tc.tile_pool(name="proj", bufs=2) as proj,  # projection weight staging
            tc.tile_pool(name="ld", bufs=3) as p_ld,
            tc.tile_pool(name="ld2", bufs=3) as p_ld2,
            tc.tile_pool(name="po", bufs=3) as p_o,
            tc.tile_pool(name="mt", bufs=3) as p_mt,
            tc.tile_pool(name="sq", bufs=3) as p_sq,
            tc.tile_pool(name="uv", bufs=2) as p_uv,
            tc.tile_pool(name="stage", bufs=2) as p_stage,
            tc.tile_pool(name="wm", bufs=2) as p_wm,   # weight-math (128,128) tiles
            tc.tile_pool(name="sm", bufs=3) as p_sm,   # tiny tiles
            tc.tile_pool(name="ps_ptr", bufs=2, space="PSUM") as ps_ptr,
            tc.tile_pool(name="ps_st", bufs=2, space="PSUM") as ps_st,
            tc.tile_pool(name="ps_mi", bufs=2, space="PSUM") as ps_mi,
        ):
            # ---- load constants ----
            ident = pers.tile([128, 128], f32, tag="ident")
            nc.sync.dma_start(ident[:], identd[:])
            ones_row = pers.tile([1, 128], f32, tag="onesr")
            nc.sync.dma_start(ones_row[:], onesrd[:])
            ones_col = pers.tile([128, 1], f32, tag="onesc")
            nc.sync.dma_start(ones_col[:], onescd[:])
            onesblk = pers.tile([128, 2], f32, tag="onesb")
            nc.sync.dma_start(onesblk[:], onesbd[:])
            nextm = pers.tile([128, 128], f32, tag="nextm")
            nc.sync.dma_start(nextm[:], nextd[:])
            prevm = pers.tile([128, 128], f32, tag="prevm")
            nc.sync.dma_start(prevm[:], prevd[:])
            eps_col = pers.tile([128, 1], f32, tag="epsc")
            nc.vector.memset(eps_col[:], EPS)

            # ---- stage A: projections ----
            ct0 = pers.tile([128, BL], f32, tag="ct0")
            nc.sync.dma_start(ct0[:], ctrlT[0:128, :])
            ct1 = pers.tile([128, BL], f32, tag="ct1")
            nc.sync.dma_start(ct1[:], ctrlT[128:256, :])
            beba = pers.tile([128, 1], f32, tag="beba")
            nc.sync.dma_start(beba[:], bebad[:])
            b2 = pers.tile([12, 1], f32, tag="b2")
            nc.sync.dma_start(b2[:], b2d[:])
            ivs = pers.tile([1, 2], f32, tag="ivs")
            nc.sync.dma_start(ivs[:], ivd[:])

            def mm_pair(dram_w, ncols):
                wa = proj.tile([128, ncols], f32, tag="wa")
                nc.sync.dma_start(wa[:], dram_w[0:128, :])
                wb = proj.tile([128, ncols], f32, tag="wb")
                nc.sync.dma_start(wb[:], dram_w[128:256, :])
                ps = ps_mi.tile([ncols, BL], f32, tag="mi")
                nc.tensor.matmul(ps[:], wa[:], ct0[:], start=True, stop=False)
                nc.tensor.matmul(ps[:], wb[:], ct1[:], start=False, stop=True)
                return ps

            p0 = mm_pair(wc0, 128)
            kT = pers.tile([128, BL], f32, tag="kT")      # [key_r^T ; key_w^T]
            nc.scalar.copy(kT[:], p0[:])

            p1 = mm_pair(wc1, 128)
            ea_sb = pers.tile([128, BL], f32, tag="ea")   # [erase^T ; add^T]
            nc.scalar.activation(ea_sb[0:64, :], p1[0:64, :], AF.Sigmoid,
                                 bias=beba[0:64, :])
            nc.scalar.activation(ea_sb[64:128, :], p1[64:128, :], AF.Tanh,
                                 bias=beba[64:128, :])

            p2 = mm_pair(wc2, 12)
            s_sb = pers.tile([12, BL], f32, tag="ssb")
            nc.scalar.activation(s_sb[:], p2[:], AF.Identity, bias=b2[:])

            # ---- stage B: per-batch scalars ----
            # transpose keys -> (BL, 128)
            kTT = ps_mi.tile([BL, 128], f32, tag="mi")
            nc.tensor.transpose(kTT[:], kT[:], ident[:])
            k2 = pers.tile([BL, 128], f32, tag="k2")
            nc.scalar.copy(k2[:], kTT[:])
            sq2 = p_sm.tile([BL, 128], f32, tag="sq2")
            nc.scalar.square(sq2[:], k2[:])

            def inv_norm(sl):
                nk = p_sm.tile([BL, 1], f32, tag="nk")
                nc.vector.tensor_reduce(nk[:], sq2[:, sl], mybir.AxisListType.X, OP.add)
                lnk = p_sm.tile([BL, 1], f32, tag="lnk")
                nc.scalar.activation(lnk[:], nk[:], AF.Ln)
                sr = p_sm.tile([BL, 1], f32, tag="sr")
                nc.scalar.activation(sr[:], lnk[:], AF.Exp, scale=0.5)
                sre = p_sm.tile([BL, 1], f32, tag="sre")
                nc.vector.tensor_scalar(sre[:], sr[:], EPS, None, OP.add)
                ik = p_sm.tile([BL, 1], f32, tag="ik")
                nc.vector.reciprocal(ik[:], sre[:])
                return ik

            invk_r = inv_norm(slice(0, 64))
            invk_w = inv_norm(slice(64, 128))

            s2T = ps_mi.tile([BL, 12], f32, tag="mi")
            nc.tensor.transpose(s2T[:], s_sb[:], ident[0:12, 0:12])
            s2 = pers.tile([BL, 12], f32, tag="s2")
            nc.scalar.copy(s2[:], s2T[:])

            iv8 = pers.tile([BL, 2], f32, tag="iv8")
            nc.gpsimd.partition_broadcast(iv8[:], ivs[:])

            def head_scalars(cols, invk, iv_col):
                """cols = (bb, bg, bgam, s0) column indices in s2; returns (8,8) tile:
                [0]=beta*invk [1]=(1-g)*init [2]=gamma [3..5]=shift [6]=g"""
                cb, cg, cgam, cs = cols
                out = pers.tile([BL, 8], f32, tag=f"scal{iv_col}", name=f"scal{iv_col}")

                def softplus(dst, src_sl):
                    e1 = p_sm.tile([BL, 1], f32, tag="e1", name="e1")
                    nc.scalar.activation(e1[:], s2[:, src_sl], AF.Exp)
                    e1p = p_sm.tile([BL, 1], f32, tag="e1p", name="e1p")
                    nc.vector.tensor_scalar(e1p[:], e1[:], 1.0, None, OP.add)
                    nc.scalar.activation(dst, e1p[:], AF.Ln)

                sp = p_sm.tile([BL, 1], f32, tag="sp")
                softplus(sp[:], slice(cb, cb + 1))
                nc.vector.scalar_tensor_tensor(out[:, 0:1], sp[:], 1.0, invk[:],
                                               OP.add, OP.mult)
                g = p_sm.tile([BL, 1], f32, tag="g")
                nc.scalar.activation(g[:], s2[:, cg:cg + 1], AF.Sigmoid)
                nc.vector.tensor_copy(out[:, 6:7], g[:])
                omg = p_sm.tile([BL, 1], f32, tag="omg")
                nc.vector.tensor_scalar(omg[:], g[:], -1.0, 1.0, OP.mult, OP.add)
                nc.vector.tensor_scalar(out[:, 1:2], omg[:],
                                        iv8[:, iv_col:iv_col + 1], None, OP.mult)
                spg = p_sm.tile([BL, 1], f32, tag="spg")
                softplus(spg[:], slice(cgam, cgam + 1))
                nc.vector.tensor_scalar(out[:, 2:3], spg[:], 1.0, None, OP.add)
                ex = p_sm.tile([BL, 3], f32, tag="ex")
                nc.scalar.activation(ex[:], s2[:, cs:cs + 3], AF.Exp)
                se = p_sm.tile([BL, 1], f32, tag="se")
                nc.vector.tensor_reduce(se[:], ex[:], mybir.AxisListType.X, OP.add)
                rse = p_sm.tile([BL, 1], f32, tag="rse")
                nc.vector.reciprocal(rse[:], se[:])
                nc.vector.tensor_scalar(out[:, 3:6], ex[:], rse[:], None, OP.mult)
                return out

            scal_r = head_scalars((0, 1, 2, 6), invk_r, 0)
            scal_w = head_scalars((3, 4, 5, 9), invk_w, 1)
            # flatten per-batch scalar rows onto one partition for PE broadcast
            scal_row = pers.tile([1, 128], f32, tag="scalrow")
            nc.sync.dma_start(scal_row[:, 0:64], scal_r[:])
            nc.sync.dma_start(scal_row[:, 64:128], scal_w[:])

            # ---- Wk_all (128, 32): per-batch block-diag key columns ----
            WT = pers.tile([32, 128], f32, tag="WT")
            nc.vector.memset(WT[:], 0.0)
            WTv = WT[:].rearrange("(a c) f -> a c f", c=4)
            nc.sync.dma_start(WTv[:, 0, 0:64], k2[:, 0:64])      # rd_r even
            nc.sync.dma_start(WTv[:, 1, 0:64], k2[:, 64:128])    # rd_w even
            nc.sync.dma_start(WTv[:, 2, 64:128], k2[:, 0:64])    # rd_r odd
            nc.sync.dma_start(WTv[:, 3, 64:128], k2[:, 64:128])  # rd_w odd
            WkT = ps_mi.tile([128, 32], f32, tag="mi")
            nc.tensor.transpose(WkT[:], WT[:], ident[0:32, 0:32])
            Wk_all = pers.tile([128, 32], mmdt, tag="wkall")
            nc.vector.tensor_copy(Wk_all[:], WkT[:])
            onesblk_r = pers.tile([128, 2], mmdt, tag="onesbr")
            nc.vector.tensor_copy(onesblk_r[:], onesblk[:])

            # ---- erase/add broadcast tiles ----
            eaTT = ps_mi.tile([BL, 128], f32, tag="mi")
            nc.tensor.transpose(eaTT[:], ea_sb[:], ident[:])
            ea2 = pers.tile([BL, 128], f32, tag="ea2")
            nc.scalar.copy(ea2[:], eaTT[:])
            e_row = pers.tile([1, BL * 64], f32, tag="erow")
            nc.sync.dma_start(e_row[:], ea2[:, 0:64])
            a_row = pers.tile([1, BL * 64], f32, tag="arow")
            nc.sync.dma_start(a_row[:], ea2[:, 64:128])
            e_allp = pers.tile([128, BL * 64], f32, tag="eallp")
            nc.gpsimd.partition_broadcast(e_allp[:], e_row[:])
            a_allp = pers.tile([128, BL * 64], f32, tag="aallp")
            nc.gpsimd.partition_broadcast(a_allp[:], a_row[:])
            e_m = pers.tile([128, 4, BL, D], f32, tag="em")
            a_m = pers.tile([128, 4, BL, D], f32, tag="am")
            e_v = e_allp[:].rearrange("p (b d) -> p b d", b=BL)
            a_v = a_allp[:].rearrange("p (b d) -> p b d", b=BL)
            for j in range(4):
                nc.scalar.copy(e_m[:, j, :, :], e_v)
                nc.scalar.copy(a_m[:, j, :, :], a_v)

            # ---- persistent stat / weight tensors ----
            rdr = [pers.tile([128, 128], f32, tag=f"rdr{b}", name=f"rdr{b}") for b in range(BL)]
            rdw = [pers.tile([128, 128], f32, tag=f"rdw{b}", name=f"rdw{b}") for b in range(BL)]
            nrm = [pers.tile([128, 128], f32, tag=f"nrm{b}", name=f"nrm{b}") for b in range(BL)]
            wrT = pers.tile([128, BL, 128], f32, tag="wrT")
            wwT = pers.tile([128, BL, 128], f32, tag="wwT")

            # ================= pass 1 + weight math =================
            for b in range(BL):
                for t in range(T1):
                    ld = p_ld.tile([128, 16, 2, D], f32, tag="ld")
                    for h in range(2):
                        src = mem[b, t * 4096 + h * 2048:
                                  t * 4096 + (h + 1) * 2048, :]
                        nc.sync.dma_start(
                            ld[:, :, h, :],
                            src.rearrange("(q p) d -> p q d", p=128))
                    for gq in range(4):
                        ptr = ps_ptr.tile([128, 512], f32, tag="ptr")
                        for qq in range(4):
                            q = gq * 4 + qq
                            nc.tensor.transpose(
                                ptr[:, qq * 128:(qq + 1) * 128],
                                ld[:, q, :, :], ident[:])
                        memT = p_mt.tile([128, 512], mmdt, tag="mt")
                        nc.vector.tensor_copy(memT[:], ptr[:])
                        sqT = p_sq.tile([128, 512], mmdt, tag="sq")
                        nc.scalar.square(sqT[:], ptr[:])
                        st_rd = ps_st.tile([4, 512], f32, tag="st_rd")
                        st_nm = ps_st.tile([2, 512], f32, tag="st_nm")
                        nc.tensor.matmul(st_rd[:],
                                         Wk_all[:, 4 * b:4 * b + 4],
                                         memT[:], start=True, stop=True)
                        nc.tensor.matmul(st_nm[:], onesblk_r[:],
                                         sqT[:], start=True, stop=True)
                        stg = p_stage.tile([4, 512], f32, tag="stg")
                        nc.scalar.copy(stg[:], st_rd[:])
                        stgn = p_stage.tile([2, 512], f32, tag="stgn")
                        nc.scalar.copy(stgn[:], st_nm[:])
                        base = t * 32 + gq * 4
                        for row, tgt, off in ((0, rdr, 0), (1, rdw, 0),
                                              (2, rdr, 16), (3, rdw, 16)):
                            nc.sync.dma_start(
                                tgt[b][base + off:base + off + 4, :],
                                stg[row:row + 1, :])
                        for row, off in ((0, 0), (1, 16)):
                            nc.sync.dma_start(
                                nrm[b][base + off:base + off + 4, :],
                                stgn[row:row + 1, :])

                # ---- weight math for batch b ----
                lnn = p_wm.tile([128, 128], f32, tag="lnn")
                nc.scalar.activation(lnn[:], nrm[b][:], AF.Ln)
                srt = p_wm.tile([128, 128], f32, tag="srt")
                nc.scalar.activation(srt[:], lnn[:], AF.Exp, scale=0.5)
                spe = p_wm.tile([128, 128], f32, tag="spe")
                nc.vector.tensor_scalar(spe[:], srt[:], EPS, None, OP.add)
                invm = p_wm.tile([128, 128], f32, tag="invm")
                nc.vector.reciprocal(invm[:], spe[:])

                for rd_t, soff, wT_t in ((rdr[b], b * 8, wrT),
                                         (rdw[b], 64 + b * 8, wwT)):
                    scb_ps = ps_mi.tile([128, 8], f32, tag="mi")
                    nc.tensor.matmul(scb_ps[:], ones_row[:],
                                     scal_row[:, soff:soff + 8],
                                     start=True, stop=True)
                    scb = p_wm.tile([128, 8], f32, tag="scb")
                    nc.vector.tensor_copy(scb[:], scb_ps[:])

                    expo = p_wm.tile([128, 128], f32, tag="expo")
                    nc.vector.scalar_tensor_tensor(expo[:], rd_t[:], scb[:, 0:1],
                                                   invm[:], OP.mult, OP.mult)
                    zpart = p_sm.tile([128, 1], f32, tag="zpart")
                    cw = p_wm.tile([128, 128], f32, tag="cw")
                    nc.scalar.activation(cw[:], expo[:], AF.Exp, accum_out=zpart[:])
                    z1 = ps_mi.tile([1, 1], f32, tag="mi")
                    nc.tensor.matmul(z1[:], zpart[:], ones_col[:],
                                     start=True, stop=True)
                    zs = p_sm.tile([1, 1], f32, tag="zs")
                    nc.scalar.copy(zs[:], z1[:])
                    rz = p_sm.tile([1, 1], f32, tag="rz")
                    nc.vector.reciprocal(rz[:], zs[:])
                    rzb = ps_mi.tile([128, 1], f32, tag="mi")
                    nc.tensor.matmul(rzb[:], ones_row[:], rz[:],
                                     start=True, stop=True)
                    grz = p_sm.tile([128, 1], f32, tag="grz")
                    nc.vector.scalar_tensor_tensor(grz[:], scb[:, 6:7], 1.0,
                                                   rzb[:], OP.bypass, OP.mult)
                    gated = p_wm.tile([128, 128], f32, tag="gated")
                    nc.vector.tensor_scalar(gated[:], cw[:], grz[:], scb[:, 1:2],
                                            OP.mult, OP.add)
                    # circular shift: s0*g[m-1] + s1*g[m] + s2*g[m+1]
                    gp1 = ps_mi.tile([128, 1], f32, tag="mi")
                    nc.tensor.matmul(gp1[:], nextm[:], gated[:, 0:1],
                                     start=True, stop=True)
                    gm1 = ps_mi.tile([128, 1], f32, tag="mi")
                    nc.tensor.matmul(gm1[:], prevm[:], gated[:, 127:128],
                                     start=True, stop=True)
                    sha = p_wm.tile([128, 128], f32, tag="sha")
                    nc.vector.tensor_scalar(sha[:], gated[:], scb[:, 4:5], None,
                                            OP.mult)
                    shb = p_wm.tile([128, 128], f32, tag="shb")
                    nc.vector.scalar_tensor_tensor(shb[:, 0:127], gated[:, 1:128],
                                                   scb[:, 5:6], sha[:, 0:127],
                                                   OP.mult, OP.add)
                    nc.vector.scalar_tensor_tensor(shb[:, 127:128], gp1[:],
                                                   scb[:, 5:6], sha[:, 127:128],
                                                   OP.mult, OP.add)
                    shc = p_wm.tile([128, 128], f32, tag="shc")
                    nc.vector.scalar_tensor_tensor(shc[:, 1:128], gated[:, 0:127],
                                                   scb[:, 3:4], shb[:, 1:128],
                                                   OP.mult, OP.add)
                    nc.vector.scalar_tensor_tensor(shc[:, 0:1], gm1[:],
                                                   scb[:, 3:4], shb[:, 0:1],
                                                   OP.mult, OP.add)
                    # sharpening: (shc + eps) ** gamma, normalized
                    lnt = p_wm.tile([128, 128], f32, tag="lnt")
                    nc.scalar.activation(lnt[:], shc[:], AF.Ln, bias=eps_col[:])
                    spart = p_sm.tile([128, 1], f32, tag="spart")
                    pw = p_wm.tile([128, 128], f32, tag="pw")
                    nc.scalar.activation(pw[:], lnt[:], AF.Exp, scale=scb[:, 2:3],
                                         accum_out=spart[:])
                    s1p = ps_mi.tile([1, 1], f32, tag="mi")
                    nc.tensor.matmul(s1p[:], spart[:], ones_col[:],
                                     start=True, stop=True)
                    ssv = p_sm.tile([1, 1], f32, tag="ssv")
                    nc.scalar.copy(ssv[:], s1p[:])
                    sse = p_sm.tile([1, 1], f32, tag="sse")
                    nc.vector.tensor_scalar(sse[:], ssv[:], EPS, None, OP.add)
                    rs = p_sm.tile([1, 1], f32, tag="rs")
                    nc.vector.reciprocal(rs[:], sse[:])
                    rsb = ps_mi.tile([128, 1], f32, tag="mi")
                    nc.tensor.matmul(rsb[:], ones_row[:], rs[:],
                                     start=True, stop=True)
                    wfin = p_wm.tile([128, 128], f32, tag="wfin")
                    nc.vector.tensor_scalar(wfin[:], pw[:], rsb[:], None, OP.mult)
                    wt_ps = ps_ptr.tile([128, 128], f32, tag="ptr")
                    nc.tensor.transpose(wt_ps[:], wfin[:], ident[:])
                    nc.vector.tensor_copy(wT_t[:, b, :], wt_ps[:])

            if DEBUG:
                for b in range(BL):
                    nc.sync.dma_start(dbg["d_rdr"][:, b, :], rdr[b][:])
                    nc.sync.dma_start(dbg["d_rdw"][:, b, :], rdw[b][:])
                    nc.sync.dma_start(dbg["d_nrm"][:, b, :], nrm[b][:])
                nc.sync.dma_start(dbg["d_wrT"][:], wrT[:])
                nc.sync.dma_start(dbg["d_wwT"][:], wwT[:])

            # ================= pass 2 =================
            rc_ps = ps_st.tile([8, 512], f32, tag="st_nm")
            n_mm = T2 * 4
            k = 0
            for tt in range(T2):
                ld2 = p_ld2.tile([128, 4, BL, D], f32, tag="ld2")
                for jj in range(4):
                    src = mem[:, tt * 512 + jj * 128:tt * 512 + (jj + 1) * 128, :]
                    nc.sync.dma_start(
                        ld2[:, jj, :, :], src.rearrange("b p d -> p b d"))
                u = p_uv.tile([128, 4, BL, D], f32, tag="u")
                nc.vector.scalar_tensor_tensor(u[:], ld2[:], 1.0, e_m[:],
                                               OP.bypass, OP.mult)
                v = p_uv.tile([128, 4, BL, D], f32, tag="v")
                nc.vector.scalar_tensor_tensor(v[:], u[:], -1.0, a_m[:],
                                               OP.mult, OP.add)
                o = p_o.tile([128, 4, BL, D], f32, tag="o")
                for b in range(BL):
                    for jj in range(4):
                        col = tt * 4 + jj
                        nc.vector.scalar_tensor_tensor(
                            o[:, jj, b, :], v[:, jj, b, :],
                            wwT[:, b, col:col + 1], ld2[:, jj, b, :],
                            OP.mult, OP.add)
                for jj in range(4):
                    col = tt * 4 + jj
                    nc.tensor.matmul(rc_ps[:], wrT[:, :, col],
                                     ld2[:, jj, :, :],
                                     start=(k == 0), stop=(k == n_mm - 1))
                    k += 1
                for jj in range(4):
                    dst = newmem[:, tt * 512 + jj * 128:
                                 tt * 512 + (jj + 1) * 128, :]
                    nc.sync.dma_start(
                        dst.rearrange("b p d -> p b d"), o[:, jj, :, :])

            rc1 = p_sm.tile([8, 512], f32, tag="rc1")
            nc.scalar.copy(rc1[:], rc_ps[:])
            for b in range(BL):
                nc.sync.dma_start(readc_d[b:b + 1, :],
                                  rc1[b:b + 1, b * 64:(b + 1) * 64])

    nc.compile()
    return nc


_CACHE = {}


def _get_program():
    key = (USE_F32R, DEBUG)
    if key not in _CACHE:
        _CACHE[key] = build_program()
    return _CACHE[key]


def _prepare_in_maps(inputs):
    f = np.float32
    memory = np.ascontiguousarray(inputs["memory"], dtype=f)
    ctrl = np.ascontiguousarray(inputs["controller_state"], dtype=f)
    wc0 = np.ascontiguousarray(
        np.concatenate([inputs["Wk_r"], inputs["Wk_w"]], axis=1), dtype=f)
    wc1 = np.ascontiguousarray(
        np.concatenate([inputs["We"], inputs["Wa"]], axis=1), dtype=f)
    wc2 = np.ascontiguousarray(np.concatenate(
        [inputs["Wb_r"], inputs["Wg_r"], inputs["Wgam_r"],
         inputs["Wb_w"], inputs["Wg_w"], inputs["Wgam_w"],
         inputs["Ws_r"], inputs["Ws_w"]], axis=1), dtype=f)
    b2 = np.concatenate(
        [inputs["bb_r"], inputs["bg_r"], inputs["bgam_r"],
         inputs["bb_w"], inputs["bg_w"], inputs["bgam_w"],
         inputs["bs_r"], inputs["bs_w"]]).astype(f).reshape(12, 1)
    beba = np.concatenate([inputs["be"], inputs["ba"]]).astype(f).reshape(128, 1)
    iv = np.array([[inputs["init_r"][0, 0], inputs["init_w"][0, 0]]], dtype=f)

    in_maps = []
    for i in range(NCORES):
        in_maps.append({
            "mem": np.ascontiguousarray(memory[i * BL:(i + 1) * BL]),
            "ctrlT": np.ascontiguousarray(ctrl[i * BL:(i + 1) * BL].T),
            "wc0": wc0, "wc1": wc1, "wc2": wc2,
            "b2": b2, "beba": beba, "iv": iv,
        })
    return in_maps


def kernel(**inputs):
    nc = _get_program()
    in_maps = _prepare_in_maps(inputs)
    res = run_bass_kernel_spmd(nc, in_maps, core_ids=list(range(NCORES)))
    newmem = np.concatenate([r["newmem"] for r in res.results], axis=0)
    readc = np.concatenate([r["readc"] for r in res.results], axis=0)
    if DEBUG:
        kernel.debug = res.results
    return readc, newmem


# revision 21
# speedup vs baseline: 1.3110x; 1.3110x over previous
"""Trainium2 Bass kernel for nn_DifferentiableSelectCopy (NTM read/write head).

Computes, for memory (B=64, M=16384, D=64) and controller_state (B=64, C=256):
  w_r, w_w = addressing heads (content + gate + circular shift + sharpening)
  read_content = sum_m w_r[m] * memory[m, :]
  new_memory   = memory * (1 - w_w (x) erase) + w_w (x) add

Sharding: data-parallel over batch, 8 batches per NeuronCore.

Per-core structure:
  Pass 1 (stats): stream memory row-tiles, PE-transpose to D-on-partitions,
    PE matmuls against keys / ones to get rawdot (per head) and row sq-norms.
  Weight math: per (batch, head) on (128, 128) M-vector tiles.
  Pass 2: re-stream memory, fused DVE scalar_tensor_tensor ops produce
    new_memory; PE matmuls accumulate read_content.
"""
import numpy as np

import concourse.bass as bass
import concourse.tile as tile
import concourse.mybir as mybir
from concourse import bacc
from concourse.bass_utils import run_bass_kernel_spmd

f32 = mybir.dt.float32
f32r = mybir.dt.float32r
OP = mybir.AluOpType
AF = mybir.ActivationFunctionType

B, M, D, C, S = 64, 16384, 64, 256, 3
NCORES = 8
BL = B // NCORES            # 8 local batches per core
EPS = 1e-8

USE_F32R = True             # relaxed-precision PE matmuls for stats/readc
DEBUG = False               # extra DRAM outputs of intermediates

# pass-1 tiling: per (b, t): 4096 rows as (128p, 32c, 64d), row = t*4096 + c*128 + p
T1 = M // 4096              # 4
# pass-2 tiling: per tt: 512 rows/batch as (128p, 4j, 64d), row = tt*512 + jj*128 + p
T2 = M // 512               # 32


def _consts():
    ident = np.eye(128, dtype=np.float32)
    ones_row = np.ones((1, 128), dtype=np.float32)
    ones_col = np.ones((128, 1), dtype=np.float32)
    onesblk = np.zeros((128, 2), dtype=np.float32)
    onesblk[0:64, 0] = 1.0
    onesblk[64:128, 1] = 1.0
    nextmat = np.zeros((128, 128), dtype=np.float32)
    nextmat[(np.arange(128) + 1) % 128, np.arange(128)] = 1.0   # out[m'] = x[m'+1]
    prevmat = np.zeros((128, 128), dtype=np.float32)
    prevmat[(np.arange(128) - 1) % 128, np.arange(128)] = 1.0   # out[m'] = x[m'-1]
    return ident, ones_row, ones_col, onesblk, nextmat, prevmat


def build_program():
    nc = bacc.Bacc("TRN2", target_bir_lowering=False, debug=False, num_devices=NCORES)
    mmdt = f32r if USE_F32R else f32

    def cast(ap):
        return ap.bitcast(mmdt)

    # ---- DRAM I/O ----
    mem = nc.dram_tensor("mem", (BL, M, D), f32, kind="ExternalInput").ap()
    ctrlT = nc.dram_tensor("ctrlT", (C, BL), f32, kind="ExternalInput").ap()
    wc0 = nc.dram_tensor("wc0", (C, 128), f32, kind="ExternalInput").ap()  # [Wk_r|Wk_w]
    wc1 = nc.dram_tensor("wc1", (C, 128), f32, kind="ExternalInput").ap()  # [We|Wa]
    wc2 = nc.dram_tensor("wc2", (C, 12), f32, kind="ExternalInput").ap()
    b2d = nc.dram_tensor("b2", (12, 1), f32, kind="ExternalInput").ap()
    bebad = nc.dram_tensor("beba", (128, 1), f32, kind="ExternalInput").ap()
    ivd = nc.dram_tensor("iv", (1, 2), f32, kind="ExternalInput").ap()

    newmem = nc.dram_tensor("newmem", (BL, M, D), f32, kind="ExternalOutput").ap()
    readc_d = nc.dram_tensor("readc", (BL, D), f32, kind="ExternalOutput").ap()
    dbg = {}
    if DEBUG:
        for nm, shp in [("d_rdr", (128, BL, 128)), ("d_rdw", (128, BL, 128)),
                        ("d_nrm", (128, BL, 128)), ("d_wrT", (128, BL, 128)),
                        ("d_wwT", (128, BL, 128))]:
            dbg[nm] = nc.dram_tensor(nm, shp, f32, kind="ExternalOutput").ap()

    c_ident, c_ones_row, c_ones_col, c_onesblk, c_next, c_prev = _consts()
    identd = nc.inline_tensor(c_ident, "c_ident").ap()
    onesrd = nc.inline_tensor(c_ones_row, "c_onesr").ap()
    onescd = nc.inline_tensor(c_ones_col, "c_onesc").ap()
    onesbd = nc.inline_tensor(c_onesblk, "c_onesb").ap()
    nextd = nc.inline_tensor(c_next, "c_next").ap()
    prevd = nc.inline_tensor(c_prev, "c_prev").ap()

    with tile.TileContext(nc) as tc:
        with (
            tc.tile_pool(name="pers", bufs=1) as pers,  # persistent tensors
            tc.tile_pool(name="proj", bufs=2) as proj,  # projection weight staging
            tc.tile_pool(name="ld", bufs=2) as p_ld,
            tc.tile_pool(name="ld2", bufs=3) as p_ld2,
            tc.tile_pool(name="po", bufs=2) as p_o,
            tc.tile_pool(name="mt", bufs=2) as p_mt,
            tc.tile_pool(name="sq", bufs=2) as p_sq,
            tc.tile_pool(name="uv", bufs=2) as p_uv,
            tc.tile_pool(name="stage", bufs=2) as p_stage,
            tc.tile_pool(name="wm", bufs=2) as p_wm,   # weight-math (128,128) tiles
            tc.tile_pool(name="sm", bufs=2) as p_sm,   # tiny tiles
            tc.tile_pool(name="ps_ptr", bufs=2, space="PSUM") as ps_ptr,
            tc.tile_pool(name="ps_st", bufs=2, space="PSUM") as ps_st,
            tc.tile_pool(name="ps_mi", bufs=2, space="PSUM") as ps_mi,
        ):
            # ---- load constants ----
            ident = pers.tile([128, 128], f32, tag="ident")
            nc.sync.dma_start(ident[:], identd[:])
            ones_row = pers.tile([1, 128], f32, tag="onesr")
            nc.sync.dma_start(ones_row[:], onesrd[:])
            ones_col = pers.tile([128, 1], f32, tag="onesc")
            nc.sync.dma_start(ones_col[:], onescd[:])
            onesblk = pers.tile([128, 2], f32, tag="onesb")
            nc.sync.dma_start(onesblk[:], onesbd[:])
            nextm = pers.tile([128, 128], f32, tag="nextm")
            nc.sync.dma_start(nextm[:], nextd[:])
            prevm = pers.tile([128, 128], f32, tag="prevm")
            nc.sync.dma_start(prevm[:], prevd[:])
            eps_col = pers.tile([128, 1], f32, tag="epsc")
            nc.vector.memset(eps_col[:], EPS)

            # ---- stage A: projections ----
            ct0 = pers.tile([128, BL], f32, tag="ct0")
            nc.sync.dma_start(ct0[:], ctrlT[0:128, :])
            ct1 = pers.tile([128, BL], f32, tag="ct1")
            nc.sync.dma_start(ct1[:], ctrlT[128:256, :])
            beba = pers.tile([128, 1], f32, tag="beba")
            nc.sync.dma_start(beba[:], bebad[:])
            b2 = pers.tile([12, 1], f32, tag="b2")
            nc.sync.dma_start(b2[:], b2d[:])
            ivs = pers.tile([1, 2], f32, tag="ivs")
            nc.sync.dma_start(ivs[:], ivd[:])

            def mm_pair(dram_w, ncols):
                wa = proj.tile([128, ncols], f32, tag="wa")
                nc.sync.dma_start(wa[:], dram_w[0:128, :])
                wb = proj.tile([128, ncols], f32, tag="wb")
                nc.sync.dma_start(wb[:], dram_w[128:256, :])
                ps = ps_mi.tile([ncols, BL], f32, tag="mi")
                nc.tensor.matmul(ps[:], wa[:], ct0[:], start=True, stop=False)
                nc.tensor.matmul(ps[:], wb[:], ct1[:], start=False, stop=True)
                return ps

            p0 = mm_pair(wc0, 128)
            kT = pers.tile([128, BL], f32, tag="kT")      # [key_r^T ; key_w^T]
            nc.scalar.copy(kT[:], p0[:])

            p1 = mm_pair(wc1, 128)
            ea_sb = pers.tile([128, BL], f32, tag="ea")   # [erase^T ; add^T]
            nc.scalar.activation(ea_sb[0:64, :], p1[0:64, :], AF.Sigmoid,
                                 bias=beba[0:64, :])
            nc.scalar.activation(ea_sb[64:128, :], p1[64:128, :], AF.Tanh,
                                 bias=beba[64:128, :])

            p2 = mm_pair(wc2, 12)
            s_sb = pers.tile([12, BL], f32, tag="ssb")
            nc.scalar.activation(s_sb[:], p2[:], AF.Identity, bias=b2[:])

            # ---- stage B: per-batch scalars ----
            # transpose keys -> (BL, 128)
            kTT = ps_mi.tile([BL, 128], f32, tag="mi")
            nc.tensor.transpose(kTT[:], kT[:], ident[:])
            k2 = pers.tile([BL, 128], f32, tag="k2")
            nc.scalar.copy(k2[:], kTT[:])
            sq2 = p_sm.tile([BL, 128], f32, tag="sq2")
            nc.scalar.square(sq2[:], k2[:])

            def inv_norm(sl):
                nk = p_sm.tile([BL, 1], f32, tag="nk")
                nc.vector.tensor_reduce(nk[:], sq2[:, sl], mybir.AxisListType.X, OP.add)
                lnk = p_sm.tile([BL, 1], f32, tag="lnk")
                nc.scalar.activation(lnk[:], nk[:], AF.Ln)
                sr = p_sm.tile([BL, 1], f32, tag="sr")
                nc.scalar.activation(sr[:], lnk[:], AF.Exp, scale=0.5)
                sre = p_sm.tile([BL, 1], f32, tag="sre")
                nc.vector.tensor_scalar(sre[:], sr[:], EPS, None, OP.add)
                ik = p_sm.tile([BL, 1], f32, tag="ik")
                nc.vector.reciprocal(ik[:], sre[:])
                return ik

            invk_r = inv_norm(slice(0, 64))
            invk_w = inv_norm(slice(64, 128))

            s2T = ps_mi.tile([BL, 12], f32, tag="mi")
            nc.tensor.transpose(s2T[:], s_sb[:], ident[0:12, 0:12])
            s2 = pers.tile([BL, 12], f32, tag="s2")
            nc.scalar.copy(s2[:], s2T[:])

            iv8 = pers.tile([BL, 2], f32, tag="iv8")
            nc.gpsimd.partition_broadcast(iv8[:], ivs[:])

            def head_scalars(cols, invk, iv_col):
                """cols = (bb, bg, bgam, s0) column indices in s2; returns (8,8) tile:
                [0]=beta*invk [1]=(1-g)*init [2]=gamma [3..5]=shift [6]=g"""
                cb, cg, cgam, cs = cols
                out = pers.tile([BL, 8], f32, tag=f"scal{iv_col}", name=f"scal{iv_col}")

                def softplus(dst, src_sl):
                    e1 = p_sm.tile([BL, 1], f32, tag="e1", name="e1")
                    nc.scalar.activation(e1[:], s2[:, src_sl], AF.Exp)
                    e1p = p_sm.tile([BL, 1], f32, tag="e1p", name="e1p")
                    nc.vector.tensor_scalar(e1p[:], e1[:], 1.0, None, OP.add)
                    nc.scalar.activation(dst, e1p[:], AF.Ln)

                sp = p_sm.tile([BL, 1], f32, tag="sp")
                softplus(sp[:], slice(cb, cb + 1))
                nc.vector.scalar_tensor_tensor(out[:, 0:1], sp[:], 1.0, invk[:],
                                               OP.add, OP.mult)
                g = p_sm.tile([BL, 1], f32, tag="g")
                nc.scalar.activation(g[:], s2[:, cg:cg + 1], AF.Sigmoid)
                nc.vector.tensor_copy(out[:, 6:7], g[:])
                omg = p_sm.tile([BL, 1], f32, tag="omg")
                nc.vector.tensor_scalar(omg[:], g[:], -1.0, 1.0, OP.mult, OP.add)
                nc.vector.tensor_scalar(out[:, 1:2], omg[:],
                                        iv8[:, iv_col:iv_col + 1], None, OP.mult)
                spg = p_sm.tile([BL, 1], f32, tag="spg")
                softplus(spg[:], slice(cgam, cgam + 1))
                nc.vector.tensor_scalar(out[:, 2:3], spg[:], 1.0, None, OP.add)
                ex = p_sm.tile([BL, 3], f32, tag="ex")
                nc.scalar.activation(ex[:], s2[:, cs:cs + 3], AF.Exp)
                se = p_sm.tile([BL, 1], f32, tag="se")
                nc.vector.tensor_reduce(se[:], ex[:], mybir.AxisListType.X, OP.add)
                rse = p_sm.tile([BL, 1], f32, tag="rse")
                nc.vector.reciprocal(rse[:], se[:])
                nc.vector.tensor_scalar(out[:, 3:6], ex[:], rse[:], None, OP.mult)
                return out

            scal_r = head_scalars((0, 1, 2, 6), invk_r, 0)
            scal_w = head_scalars((3, 4, 5, 9), invk_w, 1)
            # flatten per-batch scalar rows onto one partition for PE broadcast
            scal_row = pers.tile([1, 128], f32, tag="scalrow")
            nc.sync.dma_start(scal_row[:, 0:64], scal_r[:])
            nc.sync.dma_start(scal_row[:, 64:128], scal_w[:])

            # ---- Wk_all (128, 32): per-batch block-diag key columns ----
            WT = pers.tile([32, 128], f32, tag="WT")
            nc.vector.memset(WT[:], 0.0)
            WTv = WT[:].rearrange("(a c) f -> a c f", c=4)
            nc.sync.dma_start(WTv[:, 0, 0:64], k2[:, 0:64])      # rd_r even
            nc.sync.dma_start(WTv[:, 1, 0:64], k2[:, 64:128])    # rd_w even
            nc.sync.dma_start(WTv[:, 2, 64:128], k2[:, 0:64])    # rd_r odd
            nc.sync.dma_start(WTv[:, 3, 64:128], k2[:, 64:128])  # rd_w odd
            WkT = ps_mi.tile([128, 32], f32, tag="mi")
            nc.tensor.transpose(WkT[:], WT[:], ident[0:32, 0:32])
            Wk_all = pers.tile([128, 32], mmdt, tag="wkall")
            nc.vector.tensor_copy(Wk_all[:], WkT[:])
            onesblk_r = pers.tile([128, 2], mmdt, tag="onesbr")
            nc.vector.tensor_copy(onesblk_r[:], onesblk[:])

            # ---- erase/add broadcast tiles ----
            eaTT = ps_mi.tile([BL, 128], f32, tag="mi")
            nc.tensor.transpose(eaTT[:], ea_sb[:], ident[:])
            ea2 = pers.tile([BL, 128], f32, tag="ea2")
            nc.scalar.copy(ea2[:], eaTT[:])
            e_row = pers.tile([1, BL * 64], f32, tag="erow")
            nc.sync.dma_start(e_row[:], ea2[:, 0:64])
            a_row = pers.tile([1, BL * 64], f32, tag="arow")
            nc.sync.dma_start(a_row[:], ea2[:, 64:128])
            e_allp = pers.tile([128, BL * 64], f32, tag="eallp")
            nc.gpsimd.partition_broadcast(e_allp[:], e_row[:])
            a_allp = pers.tile([128, BL * 64], f32, tag="aallp")
            nc.gpsimd.partition_broadcast(a_allp[:], a_row[:])
            e_m = pers.tile([128, BL, 4, D], f32, tag="em")
            a_m = pers.tile([128, BL, 4, D], f32, tag="am")
            e_v = e_allp[:].rearrange("p (b d) -> p b d", b=BL)
            a_v = a_allp[:].rearrange("p (b d) -> p b d", b=BL)
            for j in range(4):
                nc.scalar.copy(e_m[:, :, j, :], e_v)
                nc.scalar.copy(a_m[:, :, j, :], a_v)

            # ---- persistent stat / weight tensors ----
            rdr = [pers.tile([128, 128], f32, tag=f"rdr{b}", name=f"rdr{b}") for b in range(BL)]
            rdw = [pers.tile([128, 128], f32, tag=f"rdw{b}", name=f"rdw{b}") for b in range(BL)]
            nrm = [pers.tile([128, 128], f32, tag=f"nrm{b}", name=f"nrm{b}") for b in range(BL)]
            wrT = pers.tile([128, BL, 128], f32, tag="wrT")
            wwT = pers.tile([128, BL, 128], f32, tag="wwT")

            # ================= pass 1 + weight math =================
            for b in range(BL):
                for t in range(T1):
                    ld = p_ld.tile([128, 16, 2, D], f32, tag="ld")
                    for h in range(2):
                        src = mem[b, t * 4096 + h * 2048:
                                  t * 4096 + (h + 1) * 2048, :]
                        nc.sync.dma_start(
                            ld[:, :, h, :],
                            src.rearrange("(q p) d -> p q d", p=128))
                    stg = p_stage.tile([4, 2048], f32, tag="stg")
                    stgn = p_stage.tile([2, 2048], f32, tag="stgn")
                    for gq in range(4):
                        ptr = ps_ptr.tile([128, 512], f32, tag="ptr")
                        for qq in range(4):
                            q = gq * 4 + qq
                            nc.tensor.transpose(
                                ptr[:, qq * 128:(qq + 1) * 128],
                                ld[:, q, :, :], ident[:])
                        memT = p_mt.tile([128, 512], mmdt, tag="mt")
                        nc.vector.tensor_copy(memT[:], ptr[:])
                        sqT = p_sq.tile([128, 512], mmdt, tag="sq")
                        nc.scalar.square(sqT[:], ptr[:])
                        st_rd = ps_st.tile([4, 512], f32, tag="st_rd")
                        st_nm = ps_st.tile([2, 512], f32, tag="st_nm")
                        nc.tensor.matmul(st_rd[:],
                                         Wk_all[:, 4 * b:4 * b + 4],
                                         memT[:], start=True, stop=True)
                        nc.tensor.matmul(st_nm[:], onesblk_r[:],
                                         sqT[:], start=True, stop=True)
                        sl = slice(gq * 512, (gq + 1) * 512)
                        nc.scalar.copy(stg[:, sl], st_rd[:])
                        nc.scalar.copy(stgn[:, sl], st_nm[:])
                    base = t * 32
                    for row, tgt, off in ((0, rdr, 0), (1, rdw, 0),
                                          (2, rdr, 16), (3, rdw, 16)):
                        nc.sync.dma_start(
                            tgt[b][base + off:base + off + 16, :],
                            stg[row:row + 1, :])
                    for row, off in ((0, 0), (1, 16)):
                        nc.sync.dma_start(
                            nrm[b][base + off:base + off + 16, :],
                            stgn[row:row + 1, :])

                # ---- weight math for batch b ----
                lnn = p_wm.tile([128, 128], f32, tag="lnn")
                nc.scalar.activation(lnn[:], nrm[b][:], AF.Ln)
                srt = p_wm.tile([128, 128], f32, tag="srt")
                nc.scalar.activation(srt[:], lnn[:], AF.Exp, scale=0.5)
                spe = p_wm.tile([128, 128], f32, tag="spe")
                nc.vector.tensor_scalar(spe[:], srt[:], EPS, None, OP.add)
                invm = p_wm.tile([128, 128], f32, tag="invm")
                nc.vector.reciprocal(invm[:], spe[:])

                for rd_t, soff, wT_t in ((rdr[b], b * 8, wrT),
                                         (rdw[b], 64 + b * 8, wwT)):
                    scb_ps = ps_mi.tile([128, 8], f32, tag="mi")
                    nc.tensor.matmul(scb_ps[:], ones_row[:],
                                     scal_row[:, soff:soff + 8],
                                     start=True, stop=True)
                    scb = p_wm.tile([128, 8], f32, tag="scb")
                    nc.vector.tensor_copy(scb[:], scb_ps[:])

                    expo = p_wm.tile([128, 128], f32, tag="expo")
                    nc.vector.scalar_tensor_tensor(expo[:], rd_t[:], scb[:, 0:1],
                                                   invm[:], OP.mult, OP.mult)
                    zpart = p_sm.tile([128, 1], f32, tag="zpart")
                    cw = p_wm.tile([128, 128], f32, tag="cw")
                    nc.scalar.activation(cw[:], expo[:], AF.Exp, accum_out=zpart[:])
                    z1 = ps_mi.tile([1, 1], f32, tag="mi")
                    nc.tensor.matmul(z1[:], zpart[:], ones_col[:],
                                     start=True, stop=True)
                    zs = p_sm.tile([1, 1], f32, tag="zs")
                    nc.scalar.copy(zs[:], z1[:])
                    rz = p_sm.tile([1, 1], f32, tag="rz")
                    nc.vector.reciprocal(rz[:], zs[:])
                    rzb = ps_mi.tile([128, 1], f32, tag="mi")
                    nc.tensor.matmul(rzb[:], ones_row[:], rz[:],
                                     start=True, stop=True)
                    grz = p_sm.tile([128, 1], f32, tag="grz")
                    nc.vector.scalar_tensor_tensor(grz[:], scb[:, 6:7], 1.0,
                                                   rzb[:], OP.bypass, OP.mult)
                    gated = p_wm.tile([128, 128], f32, tag="gated")
                    nc.vector.tensor_scalar(gated[:], cw[:], grz[:], scb[:, 1:2],
                                            OP.mult, OP.add)
                    # circular shift: s0*g[m-1] + s1*g[m] + s2*g[m+1]
                    gp1 = ps_mi.tile([128, 1], f32, tag="mi")
                    nc.tensor.matmul(gp1[:], nextm[:], gated[:, 0:1],
                                     start=True, stop=True)
                    gm1 = ps_mi.tile([128, 1], f32, tag="mi")
                    nc.tensor.matmul(gm1[:], prevm[:], gated[:, 127:128],
                                     start=True, stop=True)
                    sha = p_wm.tile([128, 128], f32, tag="sha")
                    nc.vector.tensor_scalar(sha[:], gated[:], scb[:, 4:5], None,
                                            OP.mult)
                    shb = p_wm.tile([128, 128], f32, tag="shb")
                    nc.vector.scalar_tensor_tensor(shb[:, 0:127], gated[:, 1:128],
                                                   scb[:, 5:6], sha[:, 0:127],
                                                   OP.mult, OP.add)
                    nc.vector.scalar_tensor_tensor(shb[:, 127:128], gp1[:],
                                                   scb[:, 5:6], sha[:, 127:128],
                                                   OP.mult, OP.add)
                    shc = p_wm.tile([128, 128], f32, tag="shc")
                    nc.vector.scalar_tensor_tensor(shc[:, 1:128], gated[:, 0:127],
                                                   scb[:, 3:4], shb[:, 1:128],
                                                   OP.mult, OP.add)
                    nc.vector.scalar_tensor_tensor(shc[:, 0:1], gm1[:],
                                                   scb[:, 3:4], shb[:, 0:1],
                                                   OP.mult, OP.add)
                    # sharpening: (shc + eps) ** gamma, normalized
                    lnt = p_wm.tile([128, 128], f32, tag="lnt")
                    nc.scalar.activation(lnt[:], shc[:], AF.Ln, bias=eps_col[:])
                    spart = p_sm.tile([128, 1], f32, tag="spart")
                    pw = p_wm.tile([128, 128], f32, tag="pw")
                    nc.scalar.activation(pw[:], lnt[:], AF.Exp, scale=scb[:, 2:3],
                                         accum_out=spart[:])
                    s1p = ps_mi.tile([1, 1], f32, tag="mi")
                    nc.tensor.matmul(s1p[:], spart[:], ones_col[:],
                                     start=True, stop=True)
                    ssv = p_sm.tile([1, 1], f32, tag="ssv")
                    nc.scalar.copy(ssv[:], s1p[:])
                    sse = p_sm.tile([1, 1], f32, tag="sse")
                    nc.vector.tensor_scalar(sse[:], ssv[:], EPS, None, OP.add)
                    rs = p_sm.tile([1, 1], f32, tag="rs")
                    nc.vector.reciprocal(rs[:], sse[:])
                    rsb = ps_mi.tile([128, 1], f32, tag="mi")
                    nc.tensor.matmul(rsb[:], ones_row[:], rs[:],
                                     start=True, stop=True)
                    wfin = p_wm.tile([128, 128], f32, tag="wfin")
                    nc.vector.tensor_scalar(wfin[:], pw[:], rsb[:], None, OP.mult)
                    wt_ps = ps_ptr.tile([128, 128], f32, tag="ptr")
                    nc.tensor.transpose(wt_ps[:], wfin[:], ident[:])
                    nc.vector.tensor_copy(wT_t[:, b, :], wt_ps[:])

            if DEBUG:
                for b in range(BL):
                    nc.sync.dma_start(dbg["d_rdr"][:, b, :], rdr[b][:])
                    nc.sync.dma_start(dbg["d_rdw"][:, b, :], rdw[b][:])
                    nc.sync.dma_start(dbg["d_nrm"][:, b, :], nrm[b][:])
                nc.sync.dma_start(dbg["d_wrT"][:], wrT[:])
                nc.sync.dma_start(dbg["d_wwT"][:], wwT[:])

            # ================= pass 2 =================
            rc_ps = ps_st.tile([8, 512], f32, tag="st_nm")
            n_mm = T2 * 4
            k = 0
            for tt in range(T2):
                ld2 = p_ld2.tile([128, BL, 4, D], f32, tag="ld2")
                for jj in range(4):
                    src = mem[:, tt * 512 + jj * 128:tt * 512 + (jj + 1) * 128, :]
                    nc.sync.dma_start(
                        ld2[:, :, jj, :], src.rearrange("b p d -> p b d"))
                u = p_uv.tile([128, BL, 4, D], f32, tag="u")
                nc.vector.scalar_tensor_tensor(u[:], ld2[:], 1.0, e_m[:],
                                               OP.bypass, OP.mult)
                v = p_uv.tile([128, BL, 4, D], f32, tag="v")
                nc.vector.scalar_tensor_tensor(v[:], u[:], -1.0, a_m[:],
                                               OP.mult, OP.add)
                o = p_o.tile([128, BL, 4, D], f32, tag="o")
                for b in range(BL):
                    for jj in range(4):
                        col = tt * 4 + jj
                        nc.vector.scalar_tensor_tensor(
                            o[:, b, jj, :], v[:, b, jj, :],
                            wwT[:, b, col:col + 1], ld2[:, b, jj, :],
                            OP.mult, OP.add)
                for jj in range(4):
                    col = tt * 4 + jj
                    nc.tensor.matmul(rc_ps[:], wrT[:, :, col],
                                     ld2[:, :, jj, :],
                                     start=(k == 0), stop=(k == n_mm - 1))
                    k += 1
                for jj in range(4):
                    dst = newmem[:, tt * 512 + jj * 128:
                                 tt * 512 + (jj + 1) * 128, :]
                    nc.scalar.dma_start(
                        dst.rearrange("b p d -> p b d"), o[:, :, jj, :])

            rc1 = p_sm.tile([8, 512], f32, tag="rc1")
            nc.scalar.copy(rc1[:], rc_ps[:])
            for b in range(BL):
                nc.sync.dma_start(readc_d[b:b + 1, :],
                                  rc1[b:b + 1, b * 64:(b + 1) * 64])

    nc.compile()
    return nc


_CACHE = {}


def _get_program():
    key = (USE_F32R, DEBUG)
    if key not in _CACHE:
        _CACHE[key] = build_program()
    return _CACHE[key]


def _prepare_in_maps(inputs):
    f = np.float32
    memory = np.ascontiguousarray(inputs["memory"], dtype=f)
    ctrl = np.ascontiguousarray(inputs["controller_state"], dtype=f)
    wc0 = np.ascontiguousarray(
        np.concatenate([inputs["Wk_r"], inputs["Wk_w"]], axis=1), dtype=f)
    wc1 = np.ascontiguousarray(
        np.concatenate([inputs["We"], inputs["Wa"]], axis=1), dtype=f)
    wc2 = np.ascontiguousarray(np.concatenate(
        [inputs["Wb_r"], inputs["Wg_r"], inputs["Wgam_r"],
         inputs["Wb_w"], inputs["Wg_w"], inputs["Wgam_w"],
         inputs["Ws_r"], inputs["Ws_w"]], axis=1), dtype=f)
    b2 = np.concatenate(
        [inputs["bb_r"], inputs["bg_r"], inputs["bgam_r"],
         inputs["bb_w"], inputs["bg_w"], inputs["bgam_w"],
         inputs["bs_r"], inputs["bs_w"]]).astype(f).reshape(12, 1)
    beba = np.concatenate([inputs["be"], inputs["ba"]]).astype(f).reshape(128, 1)
    iv = np.array([[inputs["init_r"][0, 0], inputs["init_w"][0, 0]]], dtype=f)

    in_maps = []
    for i in range(NCORES):
        in_maps.append({
            "mem": np.ascontiguousarray(memory[i * BL:(i + 1) * BL]),
            "ctrlT": np.ascontiguousarray(ctrl[i * BL:(i + 1) * BL].T),
            "wc0": wc0, "wc1": wc1, "wc2": wc2,
            "b2": b2, "beba": beba, "iv": iv,
        })
    return in_maps


def kernel(**inputs):
    nc = _get_program()
    in_maps = _prepare_in_maps(inputs)
    res = run_bass_kernel_spmd(nc, in_maps, core_ids=list(range(NCORES)))
    newmem = np.concatenate([r["newmem"] for r in res.results], axis=0)
    readc = np.concatenate([r["readc"] for r in res.results], axis=0)
    if DEBUG:
        kernel.debug = res.results
    return readc, newmem


# revision 22
# speedup vs baseline: 1.5543x; 1.1856x over previous
"""Trainium2 Bass kernel for nn_DifferentiableSelectCopy (NTM read/write head).

Computes, for memory (B=64, M=16384, D=64) and controller_state (B=64, C=256):
  w_r, w_w = addressing heads (content + gate + circular shift + sharpening)
  read_content = sum_m w_r[m] * memory[m, :]
  new_memory   = memory * (1 - w_w (x) erase) + w_w (x) add

Sharding: data-parallel over batch, 8 batches per NeuronCore.

Per-core structure:
  Pass 1 (stats): stream memory row-tiles, PE-transpose to D-on-partitions,
    PE matmuls against keys / ones to get rawdot (per head) and row sq-norms.
  Weight math: per (batch, head) on (128, 128) M-vector tiles.
  Pass 2: re-stream memory, fused DVE scalar_tensor_tensor ops produce
    new_memory; PE matmuls accumulate read_content.
"""
import numpy as np

import concourse.bass as bass
import concourse.tile as tile
import concourse.mybir as mybir
from concourse import bacc
from concourse.bass_utils import run_bass_kernel_spmd

f32 = mybir.dt.float32
f32r = mybir.dt.float32r
OP = mybir.AluOpType
AF = mybir.ActivationFunctionType

B, M, D, C, S = 64, 16384, 64, 256, 3
NCORES = 8
BL = B // NCORES            # 8 local batches per core
EPS = 1e-8

USE_F32R = True             # relaxed-precision PE matmuls for stats/readc
DEBUG = False               # extra DRAM outputs of intermediates

# pass-1 tiling: per (b, t): 4096 rows as (128p, 32c, 64d), row = t*4096 + c*128 + p
T1 = M // 4096              # 4
# pass-2 tiling: per tt: 512 rows/batch as (128p, 4j, 64d), row = tt*512 + jj*128 + p
T2 = M // 512               # 32


def _consts():
    ident = np.eye(128, dtype=np.float32)
    ones_row = np.ones((1, 128), dtype=np.float32)
    ones_col = np.ones((128, 1), dtype=np.float32)
    onesblk = np.zeros((128, 2), dtype=np.float32)
    onesblk[0:64, 0] = 1.0
    onesblk[64:128, 1] = 1.0
    nextmat = np.zeros((128, 128), dtype=np.float32)
    nextmat[(np.arange(128) + 1) % 128, np.arange(128)] = 1.0   # out[m'] = x[m'+1]
    prevmat = np.zeros((128, 128), dtype=np.float32)
    prevmat[(np.arange(128) - 1) % 128, np.arange(128)] = 1.0   # out[m'] = x[m'-1]
    return ident, ones_row, ones_col, onesblk, nextmat, prevmat


def build_program():
    nc = bacc.Bacc("TRN2", target_bir_lowering=False, debug=False, num_devices=NCORES)
    mmdt = f32r if USE_F32R else f32

    def cast(ap):
        return ap.bitcast(mmdt)

    # ---- DRAM I/O ----
    mem = nc.dram_tensor("mem", (BL, M, D), f32, kind="ExternalInput").ap()
    ctrlT = nc.dram_tensor("ctrlT", (C, BL), f32, kind="ExternalInput").ap()
    wc0 = nc.dram_tensor("wc0", (C, 128), f32, kind="ExternalInput").ap()  # [Wk_r|Wk_w]
    wc1 = nc.dram_tensor("wc1", (C, 128), f32, kind="ExternalInput").ap()  # [We|Wa]
    wc2 = nc.dram_tensor("wc2", (C, 12), f32, kind="ExternalInput").ap()
    b2d = nc.dram_tensor("b2", (12, 1), f32, kind="ExternalInput").ap()
    bebad = nc.dram_tensor("beba", (128, 1), f32, kind="ExternalInput").ap()
    ivd = nc.dram_tensor("iv", (1, 2), f32, kind="ExternalInput").ap()

    newmem = nc.dram_tensor("newmem", (BL, M, D), f32, kind="ExternalOutput").ap()
    readc_d = nc.dram_tensor("readc", (BL, D), f32, kind="ExternalOutput").ap()
    dbg = {}
    if DEBUG:
        for nm, shp in [("d_rdr", (128, BL, 128)), ("d_rdw", (128, BL, 128)),
                        ("d_nrm", (128, BL, 128)), ("d_wrT", (128, BL, 128)),
                        ("d_wwT", (128, BL, 128))]:
            dbg[nm] = nc.dram_tensor(nm, shp, f32, kind="ExternalOutput").ap()

    c_ident, c_ones_row, c_ones_col, c_onesblk, c_next, c_prev = _consts()
    identd = nc.inline_tensor(c_ident, "c_ident").ap()
    onesrd = nc.inline_tensor(c_ones_row, "c_onesr").ap()
    onescd = nc.inline_tensor(c_ones_col, "c_onesc").ap()
    onesbd = nc.inline_tensor(c_onesblk, "c_onesb").ap()
    nextd = nc.inline_tensor(c_next, "c_next").ap()
    prevd = nc.inline_tensor(c_prev, "c_prev").ap()

    with tile.TileContext(nc) as tc:
        with (
            tc.tile_pool(name="pers", bufs=1) as pers,  # persistent tensors
            tc.tile_pool(name="proj", bufs=2) as proj,  # projection weight staging
            tc.tile_pool(name="ld", bufs=4) as p_ld,
            tc.tile_pool(name="ld2", bufs=3) as p_ld2,
            tc.tile_pool(name="mt", bufs=2) as p_mt,
            tc.tile_pool(name="sq", bufs=2) as p_sq,
            tc.tile_pool(name="uv", bufs=2) as p_uv,
            tc.tile_pool(name="stage", bufs=2) as p_stage,
            tc.tile_pool(name="wm", bufs=2) as p_wm,   # weight-math (128,128) tiles
            tc.tile_pool(name="sm", bufs=2) as p_sm,   # tiny tiles
            tc.tile_pool(name="ps_ptr", bufs=2, space="PSUM") as ps_ptr,
            tc.tile_pool(name="ps_st", bufs=2, space="PSUM") as ps_st,
            tc.tile_pool(name="ps_mi", bufs=2, space="PSUM") as ps_mi,
        ):
            # ---- load constants ----
            ident = pers.tile([128, 128], f32, tag="ident")
            nc.sync.dma_start(ident[:], identd[:])
            ones_row = pers.tile([1, 128], f32, tag="onesr")
            nc.sync.dma_start(ones_row[:], onesrd[:])
            ones_col = pers.tile([128, 1], f32, tag="onesc")
            nc.sync.dma_start(ones_col[:], onescd[:])
            onesblk = pers.tile([128, 2], f32, tag="onesb")
            nc.sync.dma_start(onesblk[:], onesbd[:])
            nextm = pers.tile([128, 128], f32, tag="nextm")
            nc.sync.dma_start(nextm[:], nextd[:])
            prevm = pers.tile([128, 128], f32, tag="prevm")
            nc.sync.dma_start(prevm[:], prevd[:])
            eps_col = pers.tile([128, 1], f32, tag="epsc")
            nc.vector.memset(eps_col[:], EPS)

            # ---- stage A: projections ----
            ct0 = pers.tile([128, BL], f32, tag="ct0")
            nc.sync.dma_start(ct0[:], ctrlT[0:128, :])
            ct1 = pers.tile([128, BL], f32, tag="ct1")
            nc.sync.dma_start(ct1[:], ctrlT[128:256, :])
            beba = pers.tile([128, 1], f32, tag="beba")
            nc.sync.dma_start(beba[:], bebad[:])
            b2 = pers.tile([12, 1], f32, tag="b2")
            nc.sync.dma_start(b2[:], b2d[:])
            ivs = pers.tile([1, 2], f32, tag="ivs")
            nc.sync.dma_start(ivs[:], ivd[:])

            def mm_pair(dram_w, ncols):
                wa = proj.tile([128, ncols], f32, tag="wa")
                nc.sync.dma_start(wa[:], dram_w[0:128, :])
                wb = proj.tile([128, ncols], f32, tag="wb")
                nc.sync.dma_start(wb[:], dram_w[128:256, :])
                ps = ps_mi.tile([ncols, BL], f32, tag="mi")
                nc.tensor.matmul(ps[:], wa[:], ct0[:], start=True, stop=False)
                nc.tensor.matmul(ps[:], wb[:], ct1[:], start=False, stop=True)
                return ps

            p0 = mm_pair(wc0, 128)
            kT = pers.tile([128, BL], f32, tag="kT")      # [key_r^T ; key_w^T]
            nc.scalar.copy(kT[:], p0[:])

            p1 = mm_pair(wc1, 128)
            ea_sb = pers.tile([128, BL], f32, tag="ea")   # [erase^T ; add^T]
            nc.scalar.activation(ea_sb[0:64, :], p1[0:64, :], AF.Sigmoid,
                                 bias=beba[0:64, :])
            nc.scalar.activation(ea_sb[64:128, :], p1[64:128, :], AF.Tanh,
                                 bias=beba[64:128, :])

            p2 = mm_pair(wc2, 12)
            s_sb = pers.tile([12, BL], f32, tag="ssb")
            nc.scalar.activation(s_sb[:], p2[:], AF.Identity, bias=b2[:])

            # ---- stage B: per-batch scalars ----
            # transpose keys -> (BL, 128)
            kTT = ps_mi.tile([BL, 128], f32, tag="mi")
            nc.tensor.transpose(kTT[:], kT[:], ident[:])
            k2 = pers.tile([BL, 128], f32, tag="k2")
            nc.scalar.copy(k2[:], kTT[:])
            sq2 = p_sm.tile([BL, 128], f32, tag="sq2")
            nc.scalar.square(sq2[:], k2[:])

            def inv_norm(sl):
                nk = p_sm.tile([BL, 1], f32, tag="nk")
                nc.vector.tensor_reduce(nk[:], sq2[:, sl], mybir.AxisListType.X, OP.add)
                lnk = p_sm.tile([BL, 1], f32, tag="lnk")
                nc.scalar.activation(lnk[:], nk[:], AF.Ln)
                ik = p_sm.tile([BL, 1], f32, tag="ik")
                nc.scalar.activation(ik[:], lnk[:], AF.Exp, scale=-0.5)
                return ik

            invk_r = inv_norm(slice(0, 64))
            invk_w = inv_norm(slice(64, 128))

            s2T = ps_mi.tile([BL, 12], f32, tag="mi")
            nc.tensor.transpose(s2T[:], s_sb[:], ident[0:12, 0:12])
            s2 = pers.tile([BL, 12], f32, tag="s2")
            nc.scalar.copy(s2[:], s2T[:])

            iv8 = pers.tile([BL, 2], f32, tag="iv8")
            nc.gpsimd.partition_broadcast(iv8[:], ivs[:])

            def head_scalars(cols, invk, iv_col):
                """cols = (bb, bg, bgam, s0) column indices in s2; returns (8,8) tile:
                [0]=beta*invk [1]=(1-g)*init [2]=gamma [3..5]=shift [6]=g"""
                cb, cg, cgam, cs = cols
                out = pers.tile([BL, 8], f32, tag=f"scal{iv_col}", name=f"scal{iv_col}")

                def softplus(dst, src_sl):
                    e1 = p_sm.tile([BL, 1], f32, tag="e1", name="e1")
                    nc.scalar.activation(e1[:], s2[:, src_sl], AF.Exp)
                    e1p = p_sm.tile([BL, 1], f32, tag="e1p", name="e1p")
                    nc.vector.tensor_scalar(e1p[:], e1[:], 1.0, None, OP.add)
                    nc.scalar.activation(dst, e1p[:], AF.Ln)

                sp = p_sm.tile([BL, 1], f32, tag="sp")
                softplus(sp[:], slice(cb, cb + 1))
                nc.vector.scalar_tensor_tensor(out[:, 0:1], sp[:], 1.0, invk[:],
                                               OP.add, OP.mult)
                g = p_sm.tile([BL, 1], f32, tag="g")
                nc.scalar.activation(g[:], s2[:, cg:cg + 1], AF.Sigmoid)
                nc.vector.tensor_copy(out[:, 6:7], g[:])
                omg = p_sm.tile([BL, 1], f32, tag="omg")
                nc.vector.tensor_scalar(omg[:], g[:], -1.0, 1.0, OP.mult, OP.add)
                nc.vector.tensor_scalar(out[:, 1:2], omg[:],
                                        iv8[:, iv_col:iv_col + 1], None, OP.mult)
                spg = p_sm.tile([BL, 1], f32, tag="spg")
                softplus(spg[:], slice(cgam, cgam + 1))
                nc.vector.tensor_scalar(out[:, 2:3], spg[:], 1.0, None, OP.add)
                ex = p_sm.tile([BL, 3], f32, tag="ex")
                nc.scalar.activation(ex[:], s2[:, cs:cs + 3], AF.Exp)
                se = p_sm.tile([BL, 1], f32, tag="se")
                nc.vector.tensor_reduce(se[:], ex[:], mybir.AxisListType.X, OP.add)
                rse = p_sm.tile([BL, 1], f32, tag="rse")
                nc.vector.reciprocal(rse[:], se[:])
                nc.vector.tensor_scalar(out[:, 3:6], ex[:], rse[:], None, OP.mult)
                return out

            scal_r = head_scalars((0, 1, 2, 6), invk_r, 0)
            scal_w = head_scalars((3, 4, 5, 9), invk_w, 1)
            # flatten per-batch scalar rows onto one partition for PE broadcast
            scal_row = pers.tile([1, 128], f32, tag="scalrow")
            nc.sync.dma_start(scal_row[:, 0:64], scal_r[:])
            nc.sync.dma_start(scal_row[:, 64:128], scal_w[:])

            # ---- Wk_all (128, 32): per-batch block-diag key columns ----
            WT = pers.tile([32, 128], f32, tag="WT")
            nc.vector.memset(WT[:], 0.0)
            WTv = WT[:].rearrange("(a c) f -> a c f", c=4)
            nc.sync.dma_start(WTv[:, 0, 0:64], k2[:, 0:64])      # rd_r even
            nc.sync.dma_start(WTv[:, 1, 0:64], k2[:, 64:128])    # rd_w even
            nc.sync.dma_start(WTv[:, 2, 64:128], k2[:, 0:64])    # rd_r odd
            nc.sync.dma_start(WTv[:, 3, 64:128], k2[:, 64:128])  # rd_w odd
            WkT = ps_mi.tile([128, 32], f32, tag="mi")
            nc.tensor.transpose(WkT[:], WT[:], ident[0:32, 0:32])
            Wk_all = pers.tile([128, 32], mmdt, tag="wkall")
            nc.vector.tensor_copy(Wk_all[:], WkT[:])
            onesblk_r = pers.tile([128, 2], mmdt, tag="onesbr")
            nc.vector.tensor_copy(onesblk_r[:], onesblk[:])

            # ---- erase/add broadcast tiles ----
            eaTT = ps_mi.tile([BL, 128], f32, tag="mi")
            nc.tensor.transpose(eaTT[:], ea_sb[:], ident[:])
            ea2 = pers.tile([BL, 128], f32, tag="ea2")
            nc.scalar.copy(ea2[:], eaTT[:])
            e_row = pers.tile([1, BL * 64], f32, tag="erow")
            nc.sync.dma_start(e_row[:], ea2[:, 0:64])
            a_row = pers.tile([1, BL * 64], f32, tag="arow")
            nc.sync.dma_start(a_row[:], ea2[:, 64:128])
            e_allp = pers.tile([128, BL * 64], f32, tag="eallp")
            nc.gpsimd.partition_broadcast(e_allp[:], e_row[:])
            a_allp = pers.tile([128, BL * 64], f32, tag="aallp")
            nc.gpsimd.partition_broadcast(a_allp[:], a_row[:])
            e_m = pers.tile([128, BL, 4, D], f32, tag="em")
            a_m = pers.tile([128, BL, 4, D], f32, tag="am")
            e_v = e_allp[:].rearrange("p (b d) -> p b d", b=BL)
            a_v = a_allp[:].rearrange("p (b d) -> p b d", b=BL)
            for j in range(4):
                nc.scalar.copy(e_m[:, :, j, :], e_v)
                nc.scalar.copy(a_m[:, :, j, :], a_v)

            # ---- persistent stat / weight tensors ----
            rdr = [pers.tile([128, 128], f32, tag=f"rdr{b}", name=f"rdr{b}") for b in range(BL)]
            rdw = [pers.tile([128, 128], f32, tag=f"rdw{b}", name=f"rdw{b}") for b in range(BL)]
            nrm = [pers.tile([128, 128], f32, tag=f"nrm{b}", name=f"nrm{b}") for b in range(BL)]
            wrT = pers.tile([128, BL, 128], f32, tag="wrT")
            wwT = pers.tile([128, BL, 128], f32, tag="wwT")

            # ================= pass 1 + weight math =================
            for b in range(BL):
                for t in range(T1):
                    ld = p_ld.tile([128, 16, 2, D], f32, tag="ld")
                    for h in range(2):
                        for cc in range(2):
                            src = mem[b, t * 4096 + h * 2048 + cc * 1024:
                                      t * 4096 + h * 2048 + (cc + 1) * 1024, :]
                            nc.sync.dma_start(
                                ld[:, cc * 8:(cc + 1) * 8, h, :],
                                src.rearrange("(q p) d -> p q d", p=128))
                    stg = p_stage.tile([4, 2048], f32, tag="stg")
                    stgn = p_stage.tile([2, 2048], f32, tag="stgn")
                    for gq in range(4):
                        ptr = ps_ptr.tile([128, 512], f32, tag="ptr")
                        for qq in range(4):
                            q = gq * 4 + qq
                            nc.tensor.transpose(
                                ptr[:, qq * 128:(qq + 1) * 128],
                                ld[:, q, :, :], ident[:])
                        memT = p_mt.tile([128, 512], mmdt, tag="mt")
                        nc.vector.tensor_copy(memT[:], ptr[:])
                        sqT = p_sq.tile([128, 512], mmdt, tag="sq")
                        nc.scalar.square(sqT[:], ptr[:])
                        st_rd = ps_st.tile([4, 512], f32, tag="st_rd")
                        st_nm = ps_st.tile([2, 512], f32, tag="st_nm")
                        nc.tensor.matmul(st_rd[:],
                                         Wk_all[:, 4 * b:4 * b + 4],
                                         memT[:], start=True, stop=True)
                        nc.tensor.matmul(st_nm[:], onesblk_r[:],
                                         sqT[:], start=True, stop=True)
                        sl = slice(gq * 512, (gq + 1) * 512)
                        nc.vector.tensor_copy(stg[:, sl], st_rd[:])
                        nc.scalar.copy(stgn[:, sl], st_nm[:])
                    base = t * 32
                    for row, tgt, off in ((0, rdr, 0), (1, rdw, 0),
                                          (2, rdr, 16), (3, rdw, 16)):
                        nc.sync.dma_start(
                            tgt[b][base + off:base + off + 16, :],
                            stg[row:row + 1, :])
                    for row, off in ((0, 0), (1, 16)):
                        nc.sync.dma_start(
                            nrm[b][base + off:base + off + 16, :],
                            stgn[row:row + 1, :])

                # ---- weight math for batch b ----
                lnn = p_wm.tile([128, 128], f32, tag="lnn")
                nc.scalar.activation(lnn[:], nrm[b][:], AF.Ln)
                invm = p_wm.tile([128, 128], f32, tag="invm")
                nc.scalar.activation(invm[:], lnn[:], AF.Exp, scale=-0.5)

                for rd_t, soff, wT_t in ((rdr[b], b * 8, wrT),
                                         (rdw[b], 64 + b * 8, wwT)):
                    scb_ps = ps_mi.tile([128, 8], f32, tag="mi")
                    nc.tensor.matmul(scb_ps[:], ones_row[:],
                                     scal_row[:, soff:soff + 8],
                                     start=True, stop=True)
                    scb = p_wm.tile([128, 8], f32, tag="scb")
                    nc.vector.tensor_copy(scb[:], scb_ps[:])

                    expo = p_wm.tile([128, 128], f32, tag="expo")
                    nc.vector.scalar_tensor_tensor(expo[:], rd_t[:], scb[:, 0:1],
                                                   invm[:], OP.mult, OP.mult)
                    zpart = p_sm.tile([128, 1], f32, tag="zpart")
                    cw = p_wm.tile([128, 128], f32, tag="cw")
                    nc.scalar.activation(cw[:], expo[:], AF.Exp, accum_out=zpart[:])
                    z1 = ps_mi.tile([1, 1], f32, tag="mi")
                    nc.tensor.matmul(z1[:], zpart[:], ones_col[:],
                                     start=True, stop=True)
                    zs = p_sm.tile([1, 1], f32, tag="zs")
                    nc.scalar.copy(zs[:], z1[:])
                    rz = p_sm.tile([1, 1], f32, tag="rz")
                    nc.vector.reciprocal(rz[:], zs[:])
                    rzb = ps_mi.tile([128, 1], f32, tag="mi")
                    nc.tensor.matmul(rzb[:], ones_row[:], rz[:],
                                     start=True, stop=True)
                    grz = p_sm.tile([128, 1], f32, tag="grz")
                    nc.vector.scalar_tensor_tensor(grz[:], scb[:, 6:7], 1.0,
                                                   rzb[:], OP.bypass, OP.mult)
                    gated = p_wm.tile([128, 128], f32, tag="gated")
                    nc.vector.tensor_scalar(gated[:], cw[:], grz[:], scb[:, 1:2],
                                            OP.mult, OP.add)
                    # circular shift: s0*g[m-1] + s1*g[m] + s2*g[m+1]
                    gp1 = ps_mi.tile([128, 1], f32, tag="mi")
                    nc.tensor.matmul(gp1[:], nextm[:], gated[:, 0:1],
                                     start=True, stop=True)
                    gm1 = ps_mi.tile([128, 1], f32, tag="mi")
                    nc.tensor.matmul(gm1[:], prevm[:], gated[:, 127:128],
                                     start=True, stop=True)
                    sha = p_wm.tile([128, 128], f32, tag="sha")
                    nc.vector.tensor_scalar(sha[:], gated[:], scb[:, 4:5], None,
                                            OP.mult)
                    shb = p_wm.tile([128, 128], f32, tag="shb")
                    nc.vector.scalar_tensor_tensor(shb[:, 0:127], gated[:, 1:128],
                                                   scb[:, 5:6], sha[:, 0:127],
                                                   OP.mult, OP.add)
                    nc.vector.scalar_tensor_tensor(shb[:, 127:128], gp1[:],
                                                   scb[:, 5:6], sha[:, 127:128],
                                                   OP.mult, OP.add)
                    shc = p_wm.tile([128, 128], f32, tag="shc")
                    nc.vector.scalar_tensor_tensor(shc[:, 1:128], gated[:, 0:127],
                                                   scb[:, 3:4], shb[:, 1:128],
                                                   OP.mult, OP.add)
                    nc.vector.scalar_tensor_tensor(shc[:, 0:1], gm1[:],
                                                   scb[:, 3:4], shb[:, 0:1],
                                                   OP.mult, OP.add)
                    # sharpening: (shc + eps) ** gamma, normalized
                    lnt = p_wm.tile([128, 128], f32, tag="lnt")
                    nc.scalar.activation(lnt[:], shc[:], AF.Ln, bias=eps_col[:])
                    spart = p_sm.tile([128, 1], f32, tag="spart")
                    pw = p_wm.tile([128, 128], f32, tag="pw")
                    nc.scalar.activation(pw[:], lnt[:], AF.Exp, scale=scb[:, 2:3],
                                         accum_out=spart[:])
                    s1p = ps_mi.tile([1, 1], f32, tag="mi")
                    nc.tensor.matmul(s1p[:], spart[:], ones_col[:],
                                     start=True, stop=True)
                    ssv = p_sm.tile([1, 1], f32, tag="ssv")
                    nc.scalar.copy(ssv[:], s1p[:])
                    sse = p_sm.tile([1, 1], f32, tag="sse")
                    nc.vector.tensor_scalar(sse[:], ssv[:], EPS, None, OP.add)
                    rs = p_sm.tile([1, 1], f32, tag="rs")
                    nc.vector.reciprocal(rs[:], sse[:])
                    rsb = ps_mi.tile([128, 1], f32, tag="mi")
                    nc.tensor.matmul(rsb[:], ones_row[:], rs[:],
                                     start=True, stop=True)
                    wfin = p_wm.tile([128, 128], f32, tag="wfin")
                    nc.vector.tensor_scalar(wfin[:], pw[:], rsb[:], None, OP.mult)
                    wt_ps = ps_ptr.tile([128, 128], f32, tag="ptr")
                    nc.tensor.transpose(wt_ps[:], wfin[:], ident[:])
                    nc.vector.tensor_copy(wT_t[:, b, :], wt_ps[:])

            if DEBUG:
                for b in range(BL):
                    nc.sync.dma_start(dbg["d_rdr"][:, b, :], rdr[b][:])
                    nc.sync.dma_start(dbg["d_rdw"][:, b, :], rdw[b][:])
                    nc.sync.dma_start(dbg["d_nrm"][:, b, :], nrm[b][:])
                nc.sync.dma_start(dbg["d_wrT"][:], wrT[:])
                nc.sync.dma_start(dbg["d_wwT"][:], wwT[:])

            # ================= pass 2 =================
            rc_ps = ps_st.tile([8, 512], f32, tag="st_nm")
            n_mm = T2 * 4
            k = 0
            for tt in range(T2):
                ld2 = p_ld2.tile([128, BL, 4, D], f32, tag="ld2")
                for jj in range(4):
                    src = mem[:, tt * 512 + jj * 128:tt * 512 + (jj + 1) * 128, :]
                    nc.sync.dma_start(
                        ld2[:, :, jj, :], src.rearrange("b p d -> p b d"))
                u = p_uv.tile([128, BL, 4, D], f32, tag="u")
                nc.vector.scalar_tensor_tensor(u[:], ld2[:], 1.0, e_m[:],
                                               OP.bypass, OP.mult)
                nc.vector.scalar_tensor_tensor(u[:], u[:], -1.0, a_m[:],
                                               OP.mult, OP.add)
                o = u
                for b in range(BL):
                    for jj in range(4):
                        col = tt * 4 + jj
                        nc.vector.scalar_tensor_tensor(
                            o[:, b, jj, :], u[:, b, jj, :],
                            wwT[:, b, col:col + 1], ld2[:, b, jj, :],
                            OP.mult, OP.add)
                for jj in range(4):
                    col = tt * 4 + jj
                    nc.tensor.matmul(rc_ps[:], wrT[:, :, col],
                                     ld2[:, :, jj, :],
                                     start=(k == 0), stop=(k == n_mm - 1))
                    k += 1
                for jj in range(4):
                    dst = newmem[:, tt * 512 + jj * 128:
                                 tt * 512 + (jj + 1) * 128, :]
                    nc.scalar.dma_start(
                        dst.rearrange("b p d -> p b d"), o[:, :, jj, :])

            rc1 = p_sm.tile([8, 512], f32, tag="rc1")
            nc.scalar.copy(rc1[:], rc_ps[:])
            for b in range(BL):
                nc.sync.dma_start(readc_d[b:b + 1, :],
                                  rc1[b:b + 1, b * 64:(b + 1) * 64])

    nc.compile()
    return nc


_CACHE = {}


def _get_program():
    key = (USE_F32R, DEBUG)
    if key not in _CACHE:
        _CACHE[key] = build_program()
    return _CACHE[key]


def _prepare_in_maps(inputs):
    f = np.float32
    memory = np.ascontiguousarray(inputs["memory"], dtype=f)
    ctrl = np.ascontiguousarray(inputs["controller_state"], dtype=f)
    wc0 = np.ascontiguousarray(
        np.concatenate([inputs["Wk_r"], inputs["Wk_w"]], axis=1), dtype=f)
    wc1 = np.ascontiguousarray(
        np.concatenate([inputs["We"], inputs["Wa"]], axis=1), dtype=f)
    wc2 = np.ascontiguousarray(np.concatenate(
        [inputs["Wb_r"], inputs["Wg_r"], inputs["Wgam_r"],
         inputs["Wb_w"], inputs["Wg_w"], inputs["Wgam_w"],
         inputs["Ws_r"], inputs["Ws_w"]], axis=1), dtype=f)
    b2 = np.concatenate(
        [inputs["bb_r"], inputs["bg_r"], inputs["bgam_r"],
         inputs["bb_w"], inputs["bg_w"], inputs["bgam_w"],
         inputs["bs_r"], inputs["bs_w"]]).astype(f).reshape(12, 1)
    beba = np.concatenate([inputs["be"], inputs["ba"]]).astype(f).reshape(128, 1)
    iv = np.array([[inputs["init_r"][0, 0], inputs["init_w"][0, 0]]], dtype=f)

    in_maps = []
    for i in range(NCORES):
        in_maps.append({
            "mem": np.ascontiguousarray(memory[i * BL:(i + 1) * BL]),
            "ctrlT": np.ascontiguousarray(ctrl[i * BL:(i + 1) * BL].T),
            "wc0": wc0, "wc1": wc1, "wc2": wc2,
            "b2": b2, "beba": beba, "iv": iv,
        })
    return in_maps


def kernel(**inputs):
    nc = _get_program()
    in_maps = _prepare_in_maps(inputs)
    res = run_bass_kernel_spmd(nc, in_maps, core_ids=list(range(NCORES)))
    newmem = np.concatenate([r["newmem"] for r in res.results], axis=0)
    readc = np.concatenate([r["readc"] for r in res.results], axis=0)
    if DEBUG:
        kernel.debug = res.results
    return readc, newmem


# revision 25
# speedup vs baseline: 1.6747x; 1.0775x over previous
"""Trainium2 Bass kernel for nn_DifferentiableSelectCopy (NTM read/write head).

Computes, for memory (B=64, M=16384, D=64) and controller_state (B=64, C=256):
  w_r, w_w = addressing heads (content + gate + circular shift + sharpening)
  read_content = sum_m w_r[m] * memory[m, :]
  new_memory   = memory * (1 - w_w (x) erase) + w_w (x) add

Sharding: data-parallel over batch, 8 batches per NeuronCore.

Per-core structure:
  Pass 1 (stats): stream memory row-tiles, PE-transpose to D-on-partitions,
    PE matmuls against keys / ones to get rawdot (per head) and row sq-norms.
  Weight math: per (batch, head) on (128, 128) M-vector tiles.
  Pass 2: re-stream memory, fused DVE scalar_tensor_tensor ops produce
    new_memory; PE matmuls accumulate read_content.
"""
import numpy as np

import concourse.bass as bass
import concourse.tile as tile
import concourse.mybir as mybir
from concourse import bacc
from concourse.bass_utils import run_bass_kernel_spmd

f32 = mybir.dt.float32
f32r = mybir.dt.float32r
OP = mybir.AluOpType
AF = mybir.ActivationFunctionType

B, M, D, C, S = 64, 16384, 64, 256, 3
NCORES = 8
BL = B // NCORES            # 8 local batches per core
EPS = 1e-8

USE_F32R = True             # relaxed-precision PE matmuls for stats/readc
DEBUG = False               # extra DRAM outputs of intermediates

# pass-1 tiling: per (b, t): 4096 rows as (128p, 32c, 64d), row = t*4096 + c*128 + p
T1 = M // 4096              # 4
# pass-2 tiling: per tt: 512 rows/batch as (128p, 4j, 64d), row = tt*512 + jj*128 + p
T2 = M // 512               # 32


def _consts():
    ident = np.eye(128, dtype=np.float32)
    ones_row = np.ones((1, 128), dtype=np.float32)
    ones_col = np.ones((128, 1), dtype=np.float32)
    onesblk = np.zeros((128, 2), dtype=np.float32)
    onesblk[0:64, 0] = 1.0
    onesblk[64:128, 1] = 1.0
    nextmat = np.zeros((128, 128), dtype=np.float32)
    nextmat[(np.arange(128) + 1) % 128, np.arange(128)] = 1.0   # out[m'] = x[m'+1]
    prevmat = np.zeros((128, 128), dtype=np.float32)
    prevmat[(np.arange(128) - 1) % 128, np.arange(128)] = 1.0   # out[m'] = x[m'-1]
    return ident, ones_row, ones_col, onesblk, nextmat, prevmat


def build_program():
    nc = bacc.Bacc("TRN2", target_bir_lowering=False, debug=False, num_devices=NCORES)
    mmdt = f32r if USE_F32R else f32

    def cast(ap):
        return ap.bitcast(mmdt)

    # ---- DRAM I/O ----
    mem = nc.dram_tensor("mem", (BL, M, D), f32, kind="ExternalInput").ap()
    ctrlT = nc.dram_tensor("ctrlT", (C, BL), f32, kind="ExternalInput").ap()
    wc0 = nc.dram_tensor("wc0", (C, 128), f32, kind="ExternalInput").ap()  # [Wk_r|Wk_w]
    wc1 = nc.dram_tensor("wc1", (C, 128), f32, kind="ExternalInput").ap()  # [We|Wa]
    wc2 = nc.dram_tensor("wc2", (C, 12), f32, kind="ExternalInput").ap()
    b2d = nc.dram_tensor("b2", (12, 1), f32, kind="ExternalInput").ap()
    bebad = nc.dram_tensor("beba", (128, 1), f32, kind="ExternalInput").ap()
    ivd = nc.dram_tensor("iv", (1, 2), f32, kind="ExternalInput").ap()

    newmem = nc.dram_tensor("newmem", (BL, M, D), f32, kind="ExternalOutput").ap()
    readc_d = nc.dram_tensor("readc", (BL, D), f32, kind="ExternalOutput").ap()
    dbg = {}
    if DEBUG:
        for nm, shp in [("d_rdr", (128, BL, 128)), ("d_rdw", (128, BL, 128)),
                        ("d_nrm", (128, BL, 128)), ("d_wrT", (128, BL, 128)),
                        ("d_wwT", (128, BL, 128))]:
            dbg[nm] = nc.dram_tensor(nm, shp, f32, kind="ExternalOutput").ap()

    c_ident, c_ones_row, c_ones_col, c_onesblk, c_next, c_prev = _consts()
    identd = nc.inline_tensor(c_ident, "c_ident").ap()
    onesrd = nc.inline_tensor(c_ones_row, "c_onesr").ap()
    onescd = nc.inline_tensor(c_ones_col, "c_onesc").ap()
    onesbd = nc.inline_tensor(c_onesblk, "c_onesb").ap()
    nextd = nc.inline_tensor(c_next, "c_next").ap()
    prevd = nc.inline_tensor(c_prev, "c_prev").ap()

    with tile.TileContext(nc) as tc:
        with (
            tc.tile_pool(name="pers", bufs=1) as pers,  # persistent tensors
            tc.tile_pool(name="proj", bufs=2) as proj,  # projection weight staging
            tc.tile_pool(name="ld", bufs=4) as p_ld,
            tc.tile_pool(name="ld2", bufs=4) as p_ld2,
            tc.tile_pool(name="mt", bufs=2) as p_mt,
            tc.tile_pool(name="sq", bufs=2) as p_sq,
            tc.tile_pool(name="uv", bufs=3) as p_uv,
            tc.tile_pool(name="stage", bufs=2) as p_stage,
            tc.tile_pool(name="wm", bufs=2) as p_wm,   # weight-math (128,128) tiles
            tc.tile_pool(name="sm", bufs=2) as p_sm,   # tiny tiles
            tc.tile_pool(name="ps_ptr", bufs=2, space="PSUM") as ps_ptr,
            tc.tile_pool(name="ps_st", bufs=2, space="PSUM") as ps_st,
            tc.tile_pool(name="ps_mi", bufs=2, space="PSUM") as ps_mi,
        ):
            # ---- load constants ----
            ident = pers.tile([128, 128], f32, tag="ident")
            nc.sync.dma_start(ident[:], identd[:])
            ones_row = pers.tile([1, 128], f32, tag="onesr")
            nc.sync.dma_start(ones_row[:], onesrd[:])
            ones_col = pers.tile([128, 1], f32, tag="onesc")
            nc.sync.dma_start(ones_col[:], onescd[:])
            onesblk = pers.tile([128, 2], f32, tag="onesb")
            nc.sync.dma_start(onesblk[:], onesbd[:])
            nextm = pers.tile([128, 128], f32, tag="nextm")
            nc.sync.dma_start(nextm[:], nextd[:])
            prevm = pers.tile([128, 128], f32, tag="prevm")
            nc.sync.dma_start(prevm[:], prevd[:])
            eps_col = pers.tile([128, 1], f32, tag="epsc")
            nc.vector.memset(eps_col[:], EPS)

            # ---- stage A: projections ----
            ct0 = pers.tile([128, BL], f32, tag="ct0")
            nc.sync.dma_start(ct0[:], ctrlT[0:128, :])
            ct1 = pers.tile([128, BL], f32, tag="ct1")
            nc.sync.dma_start(ct1[:], ctrlT[128:256, :])
            beba = pers.tile([128, 1], f32, tag="beba")
            nc.sync.dma_start(beba[:], bebad[:])
            b2 = pers.tile([12, 1], f32, tag="b2")
            nc.sync.dma_start(b2[:], b2d[:])
            ivs = pers.tile([1, 2], f32, tag="ivs")
            nc.sync.dma_start(ivs[:], ivd[:])

            def mm_pair(dram_w, ncols):
                wa = proj.tile([128, ncols], f32, tag="wa")
                nc.sync.dma_start(wa[:], dram_w[0:128, :])
                wb = proj.tile([128, ncols], f32, tag="wb")
                nc.sync.dma_start(wb[:], dram_w[128:256, :])
                ps = ps_mi.tile([ncols, BL], f32, tag="mi")
                nc.tensor.matmul(ps[:], wa[:], ct0[:], start=True, stop=False)
                nc.tensor.matmul(ps[:], wb[:], ct1[:], start=False, stop=True)
                return ps

            p0 = mm_pair(wc0, 128)
            kT = pers.tile([128, BL], f32, tag="kT")      # [key_r^T ; key_w^T]
            nc.scalar.copy(kT[:], p0[:])

            p1 = mm_pair(wc1, 128)
            ea_sb = pers.tile([128, BL], f32, tag="ea")   # [erase^T ; add^T]
            nc.scalar.activation(ea_sb[0:64, :], p1[0:64, :], AF.Sigmoid,
                                 bias=beba[0:64, :])
            nc.scalar.activation(ea_sb[64:128, :], p1[64:128, :], AF.Tanh,
                                 bias=beba[64:128, :])

            p2 = mm_pair(wc2, 12)
            s_sb = pers.tile([12, BL], f32, tag="ssb")
            nc.scalar.activation(s_sb[:], p2[:], AF.Identity, bias=b2[:])

            # ---- stage B: per-batch scalars ----
            # transpose keys -> (BL, 128)
            kTT = ps_mi.tile([BL, 128], f32, tag="mi")
            nc.tensor.transpose(kTT[:], kT[:], ident[:])
            k2 = pers.tile([BL, 128], f32, tag="k2")
            nc.scalar.copy(k2[:], kTT[:])
            sq2 = p_sm.tile([BL, 128], f32, tag="sq2")
            nc.scalar.square(sq2[:], k2[:])

            def inv_norm(sl):
                nk = p_sm.tile([BL, 1], f32, tag="nk")
                nc.vector.tensor_reduce(nk[:], sq2[:, sl], mybir.AxisListType.X, OP.add)
                lnk = p_sm.tile([BL, 1], f32, tag="lnk")
                nc.scalar.activation(lnk[:], nk[:], AF.Ln)
                ik = p_sm.tile([BL, 1], f32, tag="ik")
                nc.scalar.activation(ik[:], lnk[:], AF.Exp, scale=-0.5)
                return ik

            invk_r = inv_norm(slice(0, 64))
            invk_w = inv_norm(slice(64, 128))

            s2T = ps_mi.tile([BL, 12], f32, tag="mi")
            nc.tensor.transpose(s2T[:], s_sb[:], ident[0:12, 0:12])
            s2 = pers.tile([BL, 12], f32, tag="s2")
            nc.scalar.copy(s2[:], s2T[:])

            iv8 = pers.tile([BL, 2], f32, tag="iv8")
            nc.gpsimd.partition_broadcast(iv8[:], ivs[:])

            def head_scalars(cols, invk, iv_col):
                """cols = (bb, bg, bgam, s0) column indices in s2; returns (8,8) tile:
                [0]=beta*invk [1]=(1-g)*init [2]=gamma [3..5]=shift [6]=g"""
                cb, cg, cgam, cs = cols
                out = pers.tile([BL, 8], f32, tag=f"scal{iv_col}", name=f"scal{iv_col}")

                def softplus(dst, src_sl):
                    e1 = p_sm.tile([BL, 1], f32, tag="e1", name="e1")
                    nc.scalar.activation(e1[:], s2[:, src_sl], AF.Exp)
                    e1p = p_sm.tile([BL, 1], f32, tag="e1p", name="e1p")
                    nc.vector.tensor_scalar(e1p[:], e1[:], 1.0, None, OP.add)
                    nc.scalar.activation(dst, e1p[:], AF.Ln)

                sp = p_sm.tile([BL, 1], f32, tag="sp")
                softplus(sp[:], slice(cb, cb + 1))
                nc.vector.scalar_tensor_tensor(out[:, 0:1], sp[:], 1.0, invk[:],
                                               OP.add, OP.mult)
                g = p_sm.tile([BL, 1], f32, tag="g")
                nc.scalar.activation(g[:], s2[:, cg:cg + 1], AF.Sigmoid)
                nc.vector.tensor_copy(out[:, 6:7], g[:])
                omg = p_sm.tile([BL, 1], f32, tag="omg")
                nc.vector.tensor_scalar(omg[:], g[:], -1.0, 1.0, OP.mult, OP.add)
                nc.vector.tensor_scalar(out[:, 1:2], omg[:],
                                        iv8[:, iv_col:iv_col + 1], None, OP.mult)
                spg = p_sm.tile([BL, 1], f32, tag="spg")
                softplus(spg[:], slice(cgam, cgam + 1))
                nc.vector.tensor_scalar(out[:, 2:3], spg[:], 1.0, None, OP.add)
                ex = p_sm.tile([BL, 3], f32, tag="ex")
                nc.scalar.activation(ex[:], s2[:, cs:cs + 3], AF.Exp)
                se = p_sm.tile([BL, 1], f32, tag="se")
                nc.vector.tensor_reduce(se[:], ex[:], mybir.AxisListType.X, OP.add)
                rse = p_sm.tile([BL, 1], f32, tag="rse")
                nc.vector.reciprocal(rse[:], se[:])
                nc.vector.tensor_scalar(out[:, 3:6], ex[:], rse[:], None, OP.mult)
                return out

            scal_r = head_scalars((0, 1, 2, 6), invk_r, 0)
            scal_w = head_scalars((3, 4, 5, 9), invk_w, 1)
            # flatten per-batch scalar rows onto one partition for PE broadcast
            scal_row = pers.tile([1, 128], f32, tag="scalrow")
            nc.sync.dma_start(scal_row[:, 0:64], scal_r[:])
            nc.sync.dma_start(scal_row[:, 64:128], scal_w[:])

            # ---- Wk_all (128, 32): per-batch block-diag key columns ----
            WT = pers.tile([32, 128], f32, tag="WT")
            nc.vector.memset(WT[:], 0.0)
            WTv = WT[:].rearrange("(a c) f -> a c f", c=4)
            nc.sync.dma_start(WTv[:, 0, 0:64], k2[:, 0:64])      # rd_r even
            nc.sync.dma_start(WTv[:, 1, 0:64], k2[:, 64:128])    # rd_w even
            nc.sync.dma_start(WTv[:, 2, 64:128], k2[:, 0:64])    # rd_r odd
            nc.sync.dma_start(WTv[:, 3, 64:128], k2[:, 64:128])  # rd_w odd
            WkT = ps_mi.tile([128, 32], f32, tag="mi")
            nc.tensor.transpose(WkT[:], WT[:], ident[0:32, 0:32])
            Wk_all = pers.tile([128, 32], mmdt, tag="wkall")
            nc.vector.tensor_copy(Wk_all[:], WkT[:])
            onesblk_r = pers.tile([128, 2], mmdt, tag="onesbr")
            nc.vector.tensor_copy(onesblk_r[:], onesblk[:])

            # ---- erase/add broadcast tiles ----
            eaTT = ps_mi.tile([BL, 128], f32, tag="mi")
            nc.tensor.transpose(eaTT[:], ea_sb[:], ident[:])
            ea2 = pers.tile([BL, 128], f32, tag="ea2")
            nc.scalar.copy(ea2[:], eaTT[:])
            e_row = pers.tile([1, BL * 64], f32, tag="erow")
            nc.sync.dma_start(e_row[:], ea2[:, 0:64])
            a_row = pers.tile([1, BL * 64], f32, tag="arow")
            nc.sync.dma_start(a_row[:], ea2[:, 64:128])
            e_allp = pers.tile([128, BL * 64], f32, tag="eallp")
            nc.gpsimd.partition_broadcast(e_allp[:], e_row[:])
            a_allp = pers.tile([128, BL * 64], f32, tag="aallp")
            nc.gpsimd.partition_broadcast(a_allp[:], a_row[:])
            e_v3 = e_allp[:].rearrange("p (b d) -> p b d", b=BL)
            a_v3 = a_allp[:].rearrange("p (b d) -> p b d", b=BL)

            # ---- persistent stat / weight tensors ----
            rdr = [pers.tile([128, 128], f32, tag=f"rdr{b}", name=f"rdr{b}") for b in range(BL)]
            rdw = [pers.tile([128, 128], f32, tag=f"rdw{b}", name=f"rdw{b}") for b in range(BL)]
            nrm = [pers.tile([128, 128], f32, tag=f"nrm{b}", name=f"nrm{b}") for b in range(BL)]
            wrT = pers.tile([128, BL, 128], f32, tag="wrT")
            wwT = pers.tile([128, BL, 128], f32, tag="wwT")

            # ================= pass 1 + weight math =================
            for b in range(BL):
                for t in range(T1):
                    ld = p_ld.tile([128, 16, 2, D], f32, tag="ld")
                    for h in range(2):
                        for cc in range(2):
                            src = mem[b, t * 4096 + h * 2048 + cc * 1024:
                                      t * 4096 + h * 2048 + (cc + 1) * 1024, :]
                            nc.sync.dma_start(
                                ld[:, cc * 8:(cc + 1) * 8, h, :],
                                src.rearrange("(q p) d -> p q d", p=128))
                    stg = p_stage.tile([4, 2048], f32, tag="stg")
                    stgn = p_stage.tile([2, 2048], f32, tag="stgn")
                    for gq in range(4):
                        ptr = ps_ptr.tile([128, 512], f32, tag="ptr")
                        for qq in range(4):
                            q = gq * 4 + qq
                            nc.tensor.transpose(
                                ptr[:, qq * 128:(qq + 1) * 128],
                                ld[:, q, :, :], ident[:])
                        memT = p_mt.tile([128, 512], mmdt, tag="mt")
                        nc.vector.tensor_copy(memT[:], ptr[:])
                        sqT = p_sq.tile([128, 512], mmdt, tag="sq")
                        nc.scalar.square(sqT[:], ptr[:])
                        st_rd = ps_st.tile([4, 512], f32, tag="st_rd")
                        st_nm = ps_st.tile([2, 512], f32, tag="st_nm")
                        nc.tensor.matmul(st_rd[:],
                                         Wk_all[:, 4 * b:4 * b + 4],
                                         memT[:], start=True, stop=True)
                        nc.tensor.matmul(st_nm[:], onesblk_r[:],
                                         sqT[:], start=True, stop=True)
                        sl = slice(gq * 512, (gq + 1) * 512)
                        nc.vector.tensor_copy(stg[:, sl], st_rd[:])
                        nc.scalar.copy(stgn[:, sl], st_nm[:])
                    base = t * 32
                    for row, tgt, off in ((0, rdr, 0), (1, rdw, 0),
                                          (2, rdr, 16), (3, rdw, 16)):
                        nc.sync.dma_start(
                            tgt[b][base + off:base + off + 16, :],
                            stg[row:row + 1, :])
                    for row, off in ((0, 0), (1, 16)):
                        nc.sync.dma_start(
                            nrm[b][base + off:base + off + 16, :],
                            stgn[row:row + 1, :])

                # ---- weight math for batch b ----
                lnn = p_wm.tile([128, 128], f32, tag="lnn")
                nc.scalar.activation(lnn[:], nrm[b][:], AF.Ln)
                invm = p_wm.tile([128, 128], f32, tag="invm")
                nc.scalar.activation(invm[:], lnn[:], AF.Exp, scale=-0.5)

                for rd_t, soff, wT_t in ((rdr[b], b * 8, wrT),
                                         (rdw[b], 64 + b * 8, wwT)):
                    scb_ps = ps_mi.tile([128, 8], f32, tag="mi")
                    nc.tensor.matmul(scb_ps[:], ones_row[:],
                                     scal_row[:, soff:soff + 8],
                                     start=True, stop=True)
                    scb = p_wm.tile([128, 8], f32, tag="scb")
                    nc.vector.tensor_copy(scb[:], scb_ps[:])

                    expo = p_wm.tile([128, 128], f32, tag="expo")
                    nc.vector.scalar_tensor_tensor(expo[:], rd_t[:], scb[:, 0:1],
                                                   invm[:], OP.mult, OP.mult)
                    zpart = p_sm.tile([128, 1], f32, tag="zpart")
                    cw = p_wm.tile([128, 128], f32, tag="cw")
                    nc.scalar.activation(cw[:], expo[:], AF.Exp, accum_out=zpart[:])
                    z1 = ps_mi.tile([1, 1], f32, tag="mi")
                    nc.tensor.matmul(z1[:], zpart[:], ones_col[:],
                                     start=True, stop=True)
                    zs = p_sm.tile([1, 1], f32, tag="zs")
                    nc.scalar.copy(zs[:], z1[:])
                    rz = p_sm.tile([1, 1], f32, tag="rz")
                    nc.vector.reciprocal(rz[:], zs[:])
                    rzb = ps_mi.tile([128, 1], f32, tag="mi")
                    nc.tensor.matmul(rzb[:], ones_row[:], rz[:],
                                     start=True, stop=True)
                    grz = p_sm.tile([128, 1], f32, tag="grz")
                    nc.vector.scalar_tensor_tensor(grz[:], scb[:, 6:7], 1.0,
                                                   rzb[:], OP.bypass, OP.mult)
                    gated = p_wm.tile([128, 128], f32, tag="gated")
                    nc.vector.tensor_scalar(gated[:], cw[:], grz[:], scb[:, 1:2],
                                            OP.mult, OP.add)
                    # circular shift: s0*g[m-1] + s1*g[m] + s2*g[m+1]
                    gp1 = ps_mi.tile([128, 1], f32, tag="mi")
                    nc.tensor.matmul(gp1[:], nextm[:], gated[:, 0:1],
                                     start=True, stop=True)
                    gm1 = ps_mi.tile([128, 1], f32, tag="mi")
                    nc.tensor.matmul(gm1[:], prevm[:], gated[:, 127:128],
                                     start=True, stop=True)
                    sha = p_wm.tile([128, 128], f32, tag="sha")
                    nc.vector.tensor_scalar(sha[:], gated[:], scb[:, 4:5], None,
                                            OP.mult)
                    shb = p_wm.tile([128, 128], f32, tag="shb")
                    nc.vector.scalar_tensor_tensor(shb[:, 0:127], gated[:, 1:128],
                                                   scb[:, 5:6], sha[:, 0:127],
                                                   OP.mult, OP.add)
                    nc.vector.scalar_tensor_tensor(shb[:, 127:128], gp1[:],
                                                   scb[:, 5:6], sha[:, 127:128],
                                                   OP.mult, OP.add)
                    shc = p_wm.tile([128, 128], f32, tag="shc")
                    nc.vector.scalar_tensor_tensor(shc[:, 1:128], gated[:, 0:127],
                                                   scb[:, 3:4], shb[:, 1:128],
                                                   OP.mult, OP.add)
                    nc.vector.scalar_tensor_tensor(shc[:, 0:1], gm1[:],
                                                   scb[:, 3:4], shb[:, 0:1],
                                                   OP.mult, OP.add)
                    # sharpening: (shc + eps) ** gamma, normalized
                    lnt = p_wm.tile([128, 128], f32, tag="lnt")
                    nc.scalar.activation(lnt[:], shc[:], AF.Ln, bias=eps_col[:])
                    spart = p_sm.tile([128, 1], f32, tag="spart")
                    pw = p_wm.tile([128, 128], f32, tag="pw")
                    nc.scalar.activation(pw[:], lnt[:], AF.Exp, scale=scb[:, 2:3],
                                         accum_out=spart[:])
                    s1p = ps_mi.tile([1, 1], f32, tag="mi")
                    nc.tensor.matmul(s1p[:], spart[:], ones_col[:],
                                     start=True, stop=True)
                    ssv = p_sm.tile([1, 1], f32, tag="ssv")
                    nc.scalar.copy(ssv[:], s1p[:])
                    sse = p_sm.tile([1, 1], f32, tag="sse")
                    nc.vector.tensor_scalar(sse[:], ssv[:], EPS, None, OP.add)
                    rs = p_sm.tile([1, 1], f32, tag="rs")
                    nc.vector.reciprocal(rs[:], sse[:])
                    rsb = ps_mi.tile([128, 1], f32, tag="mi")
                    nc.tensor.matmul(rsb[:], ones_row[:], rs[:],
                                     start=True, stop=True)
                    wfin = p_wm.tile([128, 128], f32, tag="wfin")
                    nc.vector.tensor_scalar(wfin[:], pw[:], rsb[:], None, OP.mult)
                    wt_ps = ps_ptr.tile([128, 128], f32, tag="ptr")
                    nc.tensor.transpose(wt_ps[:], wfin[:], ident[:])
                    nc.vector.tensor_copy(wT_t[:, b, :], wt_ps[:])

            if DEBUG:
                for b in range(BL):
                    nc.sync.dma_start(dbg["d_rdr"][:, b, :], rdr[b][:])
                    nc.sync.dma_start(dbg["d_rdw"][:, b, :], rdw[b][:])
                    nc.sync.dma_start(dbg["d_nrm"][:, b, :], nrm[b][:])
                nc.sync.dma_start(dbg["d_wrT"][:], wrT[:])
                nc.sync.dma_start(dbg["d_wwT"][:], wwT[:])

            # ================= pass 2 =================
            rc_ps = ps_st.tile([8, 512], f32, tag="st_nm")
            n_mm = T2 * 4
            k = 0
            for tt in range(T2):
                ld2 = p_ld2.tile([128, 4, BL, D], f32, tag="ld2")
                for jj in range(4):
                    src = mem[:, tt * 512 + jj * 128:tt * 512 + (jj + 1) * 128, :]
                    nc.sync.dma_start(
                        ld2[:, jj, :, :], src.rearrange("b p d -> p b d"))
                u = p_uv.tile([128, 4, BL, D], f32, tag="u")
                for jj in range(4):
                    col = tt * 4 + jj
                    w_bc = (wwT[:, :, col:col + 1]
                            .to_broadcast((128, BL, D)))
                    nc.vector.scalar_tensor_tensor(
                        u[:, jj], ld2[:, jj], 1.0, e_v3, OP.bypass, OP.mult)
                    nc.vector.scalar_tensor_tensor(
                        u[:, jj], u[:, jj], -1.0, a_v3, OP.mult, OP.add)
                    nc.vector.scalar_tensor_tensor(
                        u[:, jj], w_bc, 1.0, u[:, jj], OP.bypass, OP.mult)
                    nc.vector.scalar_tensor_tensor(
                        u[:, jj], u[:, jj], 1.0, ld2[:, jj], OP.bypass, OP.add)
                o = u
                for jj in range(4):
                    col = tt * 4 + jj
                    nc.tensor.matmul(rc_ps[:], wrT[:, :, col],
                                     ld2[:, jj],
                                     start=(k == 0), stop=(k == n_mm - 1))
                    k += 1
                for jj in range(4):
                    dst = newmem[:, tt * 512 + jj * 128:
                                 tt * 512 + (jj + 1) * 128, :]
                    nc.scalar.dma_start(
                        dst.rearrange("b p d -> p b d"), o[:, jj])

            rc1 = p_sm.tile([8, 512], f32, tag="rc1")
            nc.scalar.copy(rc1[:], rc_ps[:])
            for b in range(BL):
                nc.sync.dma_start(readc_d[b:b + 1, :],
                                  rc1[b:b + 1, b * 64:(b + 1) * 64])

    nc.compile()
    return nc


_CACHE = {}


def _get_program():
    key = (USE_F32R, DEBUG)
    if key not in _CACHE:
        _CACHE[key] = build_program()
    return _CACHE[key]


def _prepare_in_maps(inputs):
    f = np.float32
    memory = np.ascontiguousarray(inputs["memory"], dtype=f)
    ctrl = np.ascontiguousarray(inputs["controller_state"], dtype=f)
    wc0 = np.ascontiguousarray(
        np.concatenate([inputs["Wk_r"], inputs["Wk_w"]], axis=1), dtype=f)
    wc1 = np.ascontiguousarray(
        np.concatenate([inputs["We"], inputs["Wa"]], axis=1), dtype=f)
    wc2 = np.ascontiguousarray(np.concatenate(
        [inputs["Wb_r"], inputs["Wg_r"], inputs["Wgam_r"],
         inputs["Wb_w"], inputs["Wg_w"], inputs["Wgam_w"],
         inputs["Ws_r"], inputs["Ws_w"]], axis=1), dtype=f)
    b2 = np.concatenate(
        [inputs["bb_r"], inputs["bg_r"], inputs["bgam_r"],
         inputs["bb_w"], inputs["bg_w"], inputs["bgam_w"],
         inputs["bs_r"], inputs["bs_w"]]).astype(f).reshape(12, 1)
    beba = np.concatenate([inputs["be"], inputs["ba"]]).astype(f).reshape(128, 1)
    iv = np.array([[inputs["init_r"][0, 0], inputs["init_w"][0, 0]]], dtype=f)

    in_maps = []
    for i in range(NCORES):
        in_maps.append({
            "mem": np.ascontiguousarray(memory[i * BL:(i + 1) * BL]),
            "ctrlT": np.ascontiguousarray(ctrl[i * BL:(i + 1) * BL].T),
            "wc0": wc0, "wc1": wc1, "wc2": wc2,
            "b2": b2, "beba": beba, "iv": iv,
        })
    return in_maps


def kernel(**inputs):
    nc = _get_program()
    in_maps = _prepare_in_maps(inputs)
    res = run_bass_kernel_spmd(nc, in_maps, core_ids=list(range(NCORES)))
    newmem = np.concatenate([r["newmem"] for r in res.results], axis=0)
    readc = np.concatenate([r["readc"] for r in res.results], axis=0)
    if DEBUG:
        kernel.debug = res.results
    return readc, newmem


# revision 26
# speedup vs baseline: 1.6897x; 1.0090x over previous
"""Trainium2 Bass kernel for nn_DifferentiableSelectCopy (NTM read/write head).

Computes, for memory (B=64, M=16384, D=64) and controller_state (B=64, C=256):
  w_r, w_w = addressing heads (content + gate + circular shift + sharpening)
  read_content = sum_m w_r[m] * memory[m, :]
  new_memory   = memory * (1 - w_w (x) erase) + w_w (x) add

Sharding: data-parallel over batch, 8 batches per NeuronCore.

Per-core structure:
  Pass 1 (stats): stream memory row-tiles, PE-transpose to D-on-partitions,
    PE matmuls against keys / ones to get rawdot (per head) and row sq-norms.
  Weight math: per (batch, head) on (128, 128) M-vector tiles.
  Pass 2: re-stream memory, fused DVE scalar_tensor_tensor ops produce
    new_memory; PE matmuls accumulate read_content.
"""
import numpy as np

import concourse.bass as bass
import concourse.tile as tile
import concourse.mybir as mybir
from concourse import bacc
from concourse.bass_utils import run_bass_kernel_spmd

f32 = mybir.dt.float32
f32r = mybir.dt.float32r
OP = mybir.AluOpType
AF = mybir.ActivationFunctionType

B, M, D, C, S = 64, 16384, 64, 256, 3
NCORES = 8
BL = B // NCORES            # 8 local batches per core
EPS = 1e-8

USE_F32R = True             # relaxed-precision PE matmuls for stats/readc
DEBUG = False               # extra DRAM outputs of intermediates

# pass-1 tiling: per (b, t): 4096 rows as (128p, 32c, 64d), row = t*4096 + c*128 + p
T1 = M // 4096              # 4
# pass-2 tiling: per tt: 512 rows/batch as (128p, 4j, 64d), row = tt*512 + jj*128 + p
T2 = M // 512               # 32


def _consts():
    ident = np.eye(128, dtype=np.float32)
    ones_row = np.ones((1, 128), dtype=np.float32)
    ones_col = np.ones((128, 1), dtype=np.float32)
    onesblk = np.zeros((128, 2), dtype=np.float32)
    onesblk[0:64, 0] = 1.0
    onesblk[64:128, 1] = 1.0
    nextmat = np.zeros((128, 128), dtype=np.float32)
    nextmat[(np.arange(128) + 1) % 128, np.arange(128)] = 1.0   # out[m'] = x[m'+1]
    prevmat = np.zeros((128, 128), dtype=np.float32)
    prevmat[(np.arange(128) - 1) % 128, np.arange(128)] = 1.0   # out[m'] = x[m'-1]
    return ident, ones_row, ones_col, onesblk, nextmat, prevmat


def build_program():
    nc = bacc.Bacc("TRN2", target_bir_lowering=False, debug=False, num_devices=NCORES)
    mmdt = f32r if USE_F32R else f32

    def cast(ap):
        return ap.bitcast(mmdt)

    # ---- DRAM I/O ----
    mem = nc.dram_tensor("mem", (BL, M, D), f32, kind="ExternalInput").ap()
    ctrlT = nc.dram_tensor("ctrlT", (C, BL), f32, kind="ExternalInput").ap()
    wc0 = nc.dram_tensor("wc0", (C, 128), f32, kind="ExternalInput").ap()  # [Wk_r|Wk_w]
    wc1 = nc.dram_tensor("wc1", (C, 128), f32, kind="ExternalInput").ap()  # [We|Wa]
    wc2 = nc.dram_tensor("wc2", (C, 12), f32, kind="ExternalInput").ap()
    b2d = nc.dram_tensor("b2", (12, 1), f32, kind="ExternalInput").ap()
    bebad = nc.dram_tensor("beba", (128, 1), f32, kind="ExternalInput").ap()
    ivd = nc.dram_tensor("iv", (1, 2), f32, kind="ExternalInput").ap()

    newmem = nc.dram_tensor("newmem", (BL, M, D), f32, kind="ExternalOutput").ap()
    readc_d = nc.dram_tensor("readc", (BL, D), f32, kind="ExternalOutput").ap()
    dbg = {}
    if DEBUG:
        for nm, shp in [("d_rdr", (128, BL, 128)), ("d_rdw", (128, BL, 128)),
                        ("d_nrm", (128, BL, 128)), ("d_wrT", (128, BL, 128)),
                        ("d_wwT", (128, BL, 128))]:
            dbg[nm] = nc.dram_tensor(nm, shp, f32, kind="ExternalOutput").ap()

    c_ident, c_ones_row, c_ones_col, c_onesblk, c_next, c_prev = _consts()
    identd = nc.inline_tensor(c_ident, "c_ident").ap()
    onesrd = nc.inline_tensor(c_ones_row, "c_onesr").ap()
    onescd = nc.inline_tensor(c_ones_col, "c_onesc").ap()
    onesbd = nc.inline_tensor(c_onesblk, "c_onesb").ap()
    nextd = nc.inline_tensor(c_next, "c_next").ap()
    prevd = nc.inline_tensor(c_prev, "c_prev").ap()

    with tile.TileContext(nc) as tc:
        with (
            tc.tile_pool(name="pers", bufs=1) as pers,  # persistent tensors
            tc.tile_pool(name="proj", bufs=2) as proj,  # projection weight staging
            tc.tile_pool(name="ld", bufs=4) as p_ld,
            tc.tile_pool(name="ld2", bufs=4) as p_ld2,
            tc.tile_pool(name="mt", bufs=2) as p_mt,
            tc.tile_pool(name="sq", bufs=2) as p_sq,
            tc.tile_pool(name="uv", bufs=3) as p_uv,
            tc.tile_pool(name="stage", bufs=2) as p_stage,
            tc.tile_pool(name="wm", bufs=2) as p_wm,   # weight-math (128,128) tiles
            tc.tile_pool(name="sm", bufs=2) as p_sm,   # tiny tiles
            tc.tile_pool(name="ps_ptr", bufs=2, space="PSUM") as ps_ptr,
            tc.tile_pool(name="ps_st", bufs=2, space="PSUM") as ps_st,
            tc.tile_pool(name="ps_mi", bufs=2, space="PSUM") as ps_mi,
        ):
            # ---- load constants ----
            ident = pers.tile([128, 128], f32, tag="ident")
            nc.sync.dma_start(ident[:], identd[:])
            ones_row = pers.tile([1, 128], f32, tag="onesr")
            nc.sync.dma_start(ones_row[:], onesrd[:])
            ones_col = pers.tile([128, 1], f32, tag="onesc")
            nc.sync.dma_start(ones_col[:], onescd[:])
            onesblk = pers.tile([128, 2], f32, tag="onesb")
            nc.sync.dma_start(onesblk[:], onesbd[:])
            nextm = pers.tile([128, 128], f32, tag="nextm")
            nc.sync.dma_start(nextm[:], nextd[:])
            prevm = pers.tile([128, 128], f32, tag="prevm")
            nc.sync.dma_start(prevm[:], prevd[:])
            eps_col = pers.tile([128, 1], f32, tag="epsc")
            nc.vector.memset(eps_col[:], EPS)

            # ---- stage A: projections ----
            ct0 = pers.tile([128, BL], f32, tag="ct0")
            nc.sync.dma_start(ct0[:], ctrlT[0:128, :])
            ct1 = pers.tile([128, BL], f32, tag="ct1")
            nc.sync.dma_start(ct1[:], ctrlT[128:256, :])
            beba = pers.tile([128, 1], f32, tag="beba")
            nc.sync.dma_start(beba[:], bebad[:])
            b2 = pers.tile([12, 1], f32, tag="b2")
            nc.sync.dma_start(b2[:], b2d[:])
            ivs = pers.tile([1, 2], f32, tag="ivs")
            nc.sync.dma_start(ivs[:], ivd[:])

            def mm_pair(dram_w, ncols):
                wa = proj.tile([128, ncols], f32, tag="wa")
                nc.sync.dma_start(wa[:], dram_w[0:128, :])
                wb = proj.tile([128, ncols], f32, tag="wb")
                nc.sync.dma_start(wb[:], dram_w[128:256, :])
                ps = ps_mi.tile([ncols, BL], f32, tag="mi")
                nc.tensor.matmul(ps[:], wa[:], ct0[:], start=True, stop=False)
                nc.tensor.matmul(ps[:], wb[:], ct1[:], start=False, stop=True)
                return ps

            p0 = mm_pair(wc0, 128)
            kT = pers.tile([128, BL], f32, tag="kT")      # [key_r^T ; key_w^T]
            nc.scalar.copy(kT[:], p0[:])

            p1 = mm_pair(wc1, 128)
            ea_sb = pers.tile([128, BL], f32, tag="ea")   # [erase^T ; add^T]
            nc.scalar.activation(ea_sb[0:64, :], p1[0:64, :], AF.Sigmoid,
                                 bias=beba[0:64, :])
            nc.scalar.activation(ea_sb[64:128, :], p1[64:128, :], AF.Tanh,
                                 bias=beba[64:128, :])

            p2 = mm_pair(wc2, 12)
            s_sb = pers.tile([12, BL], f32, tag="ssb")
            nc.scalar.activation(s_sb[:], p2[:], AF.Identity, bias=b2[:])

            # ---- stage B: per-batch scalars ----
            # transpose keys -> (BL, 128)
            kTT = ps_mi.tile([BL, 128], f32, tag="mi")
            nc.tensor.transpose(kTT[:], kT[:], ident[:])
            k2 = pers.tile([BL, 128], f32, tag="k2")
            nc.scalar.copy(k2[:], kTT[:])
            sq2 = p_sm.tile([BL, 128], f32, tag="sq2")
            nc.scalar.square(sq2[:], k2[:])

            def inv_norm(sl):
                nk = p_sm.tile([BL, 1], f32, tag="nk")
                nc.vector.tensor_reduce(nk[:], sq2[:, sl], mybir.AxisListType.X, OP.add)
                lnk = p_sm.tile([BL, 1], f32, tag="lnk")
                nc.scalar.activation(lnk[:], nk[:], AF.Ln)
                ik = p_sm.tile([BL, 1], f32, tag="ik")
                nc.scalar.activation(ik[:], lnk[:], AF.Exp, scale=-0.5)
                return ik

            invk_r = inv_norm(slice(0, 64))
            invk_w = inv_norm(slice(64, 128))

            s2T = ps_mi.tile([BL, 12], f32, tag="mi")
            nc.tensor.transpose(s2T[:], s_sb[:], ident[0:12, 0:12])
            s2 = pers.tile([BL, 12], f32, tag="s2")
            nc.scalar.copy(s2[:], s2T[:])

            iv8 = pers.tile([BL, 2], f32, tag="iv8")
            nc.gpsimd.partition_broadcast(iv8[:], ivs[:])

            def head_scalars(cols, invk, iv_col):
                """cols = (bb, bg, bgam, s0) column indices in s2; returns (8,8) tile:
                [0]=beta*invk [1]=(1-g)*init [2]=gamma [3..5]=shift [6]=g"""
                cb, cg, cgam, cs = cols
                out = pers.tile([BL, 8], f32, tag=f"scal{iv_col}", name=f"scal{iv_col}")

                def softplus(dst, src_sl):
                    e1 = p_sm.tile([BL, 1], f32, tag="e1", name="e1")
                    nc.scalar.activation(e1[:], s2[:, src_sl], AF.Exp)
                    e1p = p_sm.tile([BL, 1], f32, tag="e1p", name="e1p")
                    nc.vector.tensor_scalar(e1p[:], e1[:], 1.0, None, OP.add)
                    nc.scalar.activation(dst, e1p[:], AF.Ln)

                sp = p_sm.tile([BL, 1], f32, tag="sp")
                softplus(sp[:], slice(cb, cb + 1))
                nc.vector.scalar_tensor_tensor(out[:, 0:1], sp[:], 1.0, invk[:],
                                               OP.add, OP.mult)
                g = p_sm.tile([BL, 1], f32, tag="g")
                nc.scalar.activation(g[:], s2[:, cg:cg + 1], AF.Sigmoid)
                nc.vector.tensor_copy(out[:, 6:7], g[:])
                omg = p_sm.tile([BL, 1], f32, tag="omg")
                nc.vector.tensor_scalar(omg[:], g[:], -1.0, 1.0, OP.mult, OP.add)
                nc.vector.tensor_scalar(out[:, 1:2], omg[:],
                                        iv8[:, iv_col:iv_col + 1], None, OP.mult)
                spg = p_sm.tile([BL, 1], f32, tag="spg")
                softplus(spg[:], slice(cgam, cgam + 1))
                nc.vector.tensor_scalar(out[:, 2:3], spg[:], 1.0, None, OP.add)
                ex = p_sm.tile([BL, 3], f32, tag="ex")
                nc.scalar.activation(ex[:], s2[:, cs:cs + 3], AF.Exp)
                se = p_sm.tile([BL, 1], f32, tag="se")
                nc.vector.tensor_reduce(se[:], ex[:], mybir.AxisListType.X, OP.add)
                rse = p_sm.tile([BL, 1], f32, tag="rse")
                nc.vector.reciprocal(rse[:], se[:])
                nc.vector.tensor_scalar(out[:, 3:6], ex[:], rse[:], None, OP.mult)
                return out

            scal_r = head_scalars((0, 1, 2, 6), invk_r, 0)
            scal_w = head_scalars((3, 4, 5, 9), invk_w, 1)
            # flatten per-batch scalar rows onto one partition for PE broadcast
            scal_row = pers.tile([1, 128], f32, tag="scalrow")
            nc.sync.dma_start(scal_row[:, 0:64], scal_r[:])
            nc.sync.dma_start(scal_row[:, 64:128], scal_w[:])

            # ---- Wk_all (128, 32): per-batch block-diag key columns ----
            WT = pers.tile([32, 128], f32, tag="WT")
            nc.vector.memset(WT[:], 0.0)
            WTv = WT[:].rearrange("(a c) f -> a c f", c=4)
            nc.sync.dma_start(WTv[:, 0, 0:64], k2[:, 0:64])      # rd_r even
            nc.sync.dma_start(WTv[:, 1, 0:64], k2[:, 64:128])    # rd_w even
            nc.sync.dma_start(WTv[:, 2, 64:128], k2[:, 0:64])    # rd_r odd
            nc.sync.dma_start(WTv[:, 3, 64:128], k2[:, 64:128])  # rd_w odd
            WkT = ps_mi.tile([128, 32], f32, tag="mi")
            nc.tensor.transpose(WkT[:], WT[:], ident[0:32, 0:32])
            Wk_all = pers.tile([128, 32], mmdt, tag="wkall")
            nc.vector.tensor_copy(Wk_all[:], WkT[:])
            onesblk_r = pers.tile([128, 2], mmdt, tag="onesbr")
            nc.vector.tensor_copy(onesblk_r[:], onesblk[:])

            # ---- erase/add broadcast tiles ----
            eaTT = ps_mi.tile([BL, 128], f32, tag="mi")
            nc.tensor.transpose(eaTT[:], ea_sb[:], ident[:])
            ea2 = pers.tile([BL, 128], f32, tag="ea2")
            nc.scalar.copy(ea2[:], eaTT[:])
            e_row = pers.tile([1, BL * 64], f32, tag="erow")
            nc.sync.dma_start(e_row[:], ea2[:, 0:64])
            a_row = pers.tile([1, BL * 64], f32, tag="arow")
            nc.sync.dma_start(a_row[:], ea2[:, 64:128])
            e_allp = pers.tile([128, BL * 64], f32, tag="eallp")
            nc.gpsimd.partition_broadcast(e_allp[:], e_row[:])
            a_allp = pers.tile([128, BL * 64], f32, tag="aallp")
            nc.gpsimd.partition_broadcast(a_allp[:], a_row[:])
            e_v3 = e_allp[:].rearrange("p (b d) -> p b d", b=BL)
            a_v3 = a_allp[:].rearrange("p (b d) -> p b d", b=BL)

            # ---- persistent stat / weight tensors ----
            rdr = [pers.tile([128, 128], f32, tag=f"rdr{b}", name=f"rdr{b}") for b in range(BL)]
            rdw = [pers.tile([128, 128], f32, tag=f"rdw{b}", name=f"rdw{b}") for b in range(BL)]
            nrm = [pers.tile([128, 128], f32, tag=f"nrm{b}", name=f"nrm{b}") for b in range(BL)]
            wrT = pers.tile([128, BL, 128], f32, tag="wrT")
            wwT = pers.tile([128, BL, 128], f32, tag="wwT")

            # ================= pass 1 + weight math =================
            for b in range(BL):
                for t in range(T1):
                    ld = p_ld.tile([128, 16, 2, D], f32, tag="ld")
                    for h in range(2):
                        src = mem[b, t * 4096 + h * 2048:
                                  t * 4096 + (h + 1) * 2048, :]
                        nc.sync.dma_start(
                            ld[:, :, h, :],
                            src.rearrange("(q p) d -> p q d", p=128))
                    stg = p_stage.tile([4, 2048], f32, tag="stg")
                    stgn = p_stage.tile([2, 2048], f32, tag="stgn")
                    for gq in range(4):
                        ptr = ps_ptr.tile([128, 512], f32, tag="ptr")
                        for qq in range(4):
                            q = gq * 4 + qq
                            nc.tensor.transpose(
                                ptr[:, qq * 128:(qq + 1) * 128],
                                ld[:, q, :, :], ident[:])
                        memT = p_mt.tile([128, 512], mmdt, tag="mt")
                        nc.vector.tensor_copy(memT[:], ptr[:])
                        sqT = p_sq.tile([128, 512], mmdt, tag="sq")
                        nc.scalar.square(sqT[:], ptr[:])
                        st_rd = ps_st.tile([4, 512], f32, tag="st_rd")
                        st_nm = ps_st.tile([2, 512], f32, tag="st_nm")
                        nc.tensor.matmul(st_rd[:],
                                         Wk_all[:, 4 * b:4 * b + 4],
                                         memT[:], start=True, stop=True)
                        nc.tensor.matmul(st_nm[:], onesblk_r[:],
                                         sqT[:], start=True, stop=True)
                        sl = slice(gq * 512, (gq + 1) * 512)
                        nc.vector.tensor_copy(stg[:, sl], st_rd[:])
                        nc.scalar.copy(stgn[:, sl], st_nm[:])
                    base = t * 32
                    stg_v = stg[:].rearrange("(a r) n -> a r n", r=2)
                    nc.sync.dma_start(rdr[b][base:base + 32, :],
                                      stg_v[:, 0, :])
                    nc.sync.dma_start(rdw[b][base:base + 32, :],
                                      stg_v[:, 1, :])
                    nc.sync.dma_start(nrm[b][base:base + 32, :], stgn[:])

                # ---- weight math for batch b ----
                i32 = mybir.dt.int32
                ysh = p_wm.tile([128, 128], i32, tag="ysh")
                nc.vector.tensor_scalar(ysh[:], nrm[b][:].bitcast(i32), 1, None,
                                        OP.arith_shift_right)
                y0i = p_wm.tile([128, 128], i32, tag="y0i")
                nc.vector.tensor_scalar(y0i[:], ysh[:], -1, 0x5F3759DF,
                                        OP.mult, OP.add)
                hnr = p_wm.tile([128, 128], f32, tag="hnr")
                nc.vector.tensor_scalar(hnr[:], nrm[b][:], 0.5, None, OP.mult)
                invm = p_wm.tile([128, 128], f32, tag="invm")
                yy = p_wm.tile([128, 128], f32, tag="yy")
                ycur = y0i[:].bitcast(f32)
                for _ in range(3):
                    nc.vector.tensor_tensor(yy[:], ycur, ycur, OP.mult)
                    nc.vector.tensor_tensor(yy[:], yy[:], hnr[:], OP.mult)
                    nc.vector.tensor_scalar(yy[:], yy[:], -1.0, 1.5,
                                            OP.mult, OP.add)
                    nc.vector.tensor_tensor(invm[:], ycur, yy[:], OP.mult)
                    ycur = invm[:]

                for rd_t, soff, wT_t in ((rdr[b], b * 8, wrT),
                                         (rdw[b], 64 + b * 8, wwT)):
                    scb_ps = ps_mi.tile([128, 8], f32, tag="mi")
                    nc.tensor.matmul(scb_ps[:], ones_row[:],
                                     scal_row[:, soff:soff + 8],
                                     start=True, stop=True)
                    scb = p_wm.tile([128, 8], f32, tag="scb")
                    nc.vector.tensor_copy(scb[:], scb_ps[:])

                    expo = p_wm.tile([128, 128], f32, tag="expo")
                    nc.vector.scalar_tensor_tensor(expo[:], rd_t[:], scb[:, 0:1],
                                                   invm[:], OP.mult, OP.mult)
                    zpart = p_sm.tile([128, 1], f32, tag="zpart")
                    cw = p_wm.tile([128, 128], f32, tag="cw")
                    nc.scalar.activation(cw[:], expo[:], AF.Exp, accum_out=zpart[:])
                    z1 = ps_mi.tile([1, 1], f32, tag="mi")
                    nc.tensor.matmul(z1[:], zpart[:], ones_col[:],
                                     start=True, stop=True)
                    zs = p_sm.tile([1, 1], f32, tag="zs")
                    nc.scalar.copy(zs[:], z1[:])
                    rz = p_sm.tile([1, 1], f32, tag="rz")
                    nc.vector.reciprocal(rz[:], zs[:])
                    rzb = ps_mi.tile([128, 1], f32, tag="mi")
                    nc.tensor.matmul(rzb[:], ones_row[:], rz[:],
                                     start=True, stop=True)
                    grz = p_sm.tile([128, 1], f32, tag="grz")
                    nc.vector.scalar_tensor_tensor(grz[:], scb[:, 6:7], 1.0,
                                                   rzb[:], OP.bypass, OP.mult)
                    gated = p_wm.tile([128, 128], f32, tag="gated")
                    nc.vector.tensor_scalar(gated[:], cw[:], grz[:], scb[:, 1:2],
                                            OP.mult, OP.add)
                    # circular shift: s0*g[m-1] + s1*g[m] + s2*g[m+1]
                    gp1 = ps_mi.tile([128, 1], f32, tag="mi")
                    nc.tensor.matmul(gp1[:], nextm[:], gated[:, 0:1],
                                     start=True, stop=True)
                    gm1 = ps_mi.tile([128, 1], f32, tag="mi")
                    nc.tensor.matmul(gm1[:], prevm[:], gated[:, 127:128],
                                     start=True, stop=True)
                    sha = p_wm.tile([128, 128], f32, tag="sha")
                    nc.vector.tensor_scalar(sha[:], gated[:], scb[:, 4:5], None,
                                            OP.mult)
                    shb = p_wm.tile([128, 128], f32, tag="shb")
                    nc.vector.scalar_tensor_tensor(shb[:, 0:127], gated[:, 1:128],
                                                   scb[:, 5:6], sha[:, 0:127],
                                                   OP.mult, OP.add)
                    nc.vector.scalar_tensor_tensor(shb[:, 127:128], gp1[:],
                                                   scb[:, 5:6], sha[:, 127:128],
                                                   OP.mult, OP.add)
                    shc = p_wm.tile([128, 128], f32, tag="shc")
                    nc.vector.scalar_tensor_tensor(shc[:, 1:128], gated[:, 0:127],
                                                   scb[:, 3:4], shb[:, 1:128],
                                                   OP.mult, OP.add)
                    nc.vector.scalar_tensor_tensor(shc[:, 0:1], gm1[:],
                                                   scb[:, 3:4], shb[:, 0:1],
                                                   OP.mult, OP.add)
                    # sharpening: (shc + eps) ** gamma, normalized
                    lnt = p_wm.tile([128, 128], f32, tag="lnt")
                    nc.scalar.activation(lnt[:], shc[:], AF.Ln, bias=eps_col[:])
                    spart = p_sm.tile([128, 1], f32, tag="spart")
                    pw = p_wm.tile([128, 128], f32, tag="pw")
                    nc.scalar.activation(pw[:], lnt[:], AF.Exp, scale=scb[:, 2:3],
                                         accum_out=spart[:])
                    s1p = ps_mi.tile([1, 1], f32, tag="mi")
                    nc.tensor.matmul(s1p[:], spart[:], ones_col[:],
                                     start=True, stop=True)
                    ssv = p_sm.tile([1, 1], f32, tag="ssv")
                    nc.scalar.copy(ssv[:], s1p[:])
                    sse = p_sm.tile([1, 1], f32, tag="sse")
                    nc.vector.tensor_scalar(sse[:], ssv[:], EPS, None, OP.add)
                    rs = p_sm.tile([1, 1], f32, tag="rs")
                    nc.vector.reciprocal(rs[:], sse[:])
                    rsb = ps_mi.tile([128, 1], f32, tag="mi")
                    nc.tensor.matmul(rsb[:], ones_row[:], rs[:],
                                     start=True, stop=True)
                    wfin = p_wm.tile([128, 128], f32, tag="wfin")
                    nc.vector.tensor_scalar(wfin[:], pw[:], rsb[:], None, OP.mult)
                    wt_ps = ps_ptr.tile([128, 128], f32, tag="ptr")
                    nc.tensor.transpose(wt_ps[:], wfin[:], ident[:])
                    nc.vector.tensor_copy(wT_t[:, b, :], wt_ps[:])

            if DEBUG:
                for b in range(BL):
                    nc.sync.dma_start(dbg["d_rdr"][:, b, :], rdr[b][:])
                    nc.sync.dma_start(dbg["d_rdw"][:, b, :], rdw[b][:])
                    nc.sync.dma_start(dbg["d_nrm"][:, b, :], nrm[b][:])
                nc.sync.dma_start(dbg["d_wrT"][:], wrT[:])
                nc.sync.dma_start(dbg["d_wwT"][:], wwT[:])

            # ================= pass 2 =================
            rc_ps = ps_st.tile([8, 512], f32, tag="st_nm")
            n_mm = T2 * 4
            k = 0
            for tt in range(T2):
                ld2 = p_ld2.tile([128, 4, BL, D], f32, tag="ld2")
                for jj in range(4):
                    src = mem[:, tt * 512 + jj * 128:tt * 512 + (jj + 1) * 128, :]
                    nc.sync.dma_start(
                        ld2[:, jj, :, :], src.rearrange("b p d -> p b d"))
                u = p_uv.tile([128, 4, BL, D], f32, tag="u")
                for jj in range(4):
                    col = tt * 4 + jj
                    w_bc = (wwT[:, :, col:col + 1]
                            .to_broadcast((128, BL, D)))
                    nc.vector.scalar_tensor_tensor(
                        u[:, jj], ld2[:, jj], 1.0, e_v3, OP.bypass, OP.mult)
                    nc.vector.scalar_tensor_tensor(
                        u[:, jj], u[:, jj], -1.0, a_v3, OP.mult, OP.add)
                    nc.vector.scalar_tensor_tensor(
                        u[:, jj], w_bc, 1.0, u[:, jj], OP.bypass, OP.mult)
                    nc.vector.scalar_tensor_tensor(
                        u[:, jj], u[:, jj], 1.0, ld2[:, jj], OP.bypass, OP.add)
                o = u
                for jj in range(4):
                    col = tt * 4 + jj
                    nc.tensor.matmul(rc_ps[:], wrT[:, :, col],
                                     ld2[:, jj],
                                     start=(k == 0), stop=(k == n_mm - 1))
                    k += 1
                for jj in range(4):
                    dst = newmem[:, tt * 512 + jj * 128:
                                 tt * 512 + (jj + 1) * 128, :]
                    nc.scalar.dma_start(
                        dst.rearrange("b p d -> p b d"), o[:, jj])

            rc1 = p_sm.tile([8, 512], f32, tag="rc1")
            nc.scalar.copy(rc1[:], rc_ps[:])
            for b in range(BL):
                nc.sync.dma_start(readc_d[b:b + 1, :],
                                  rc1[b:b + 1, b * 64:(b + 1) * 64])

    nc.compile()
    return nc


_CACHE = {}


def _get_program():
    key = (USE_F32R, DEBUG)
    if key not in _CACHE:
        _CACHE[key] = build_program()
    return _CACHE[key]


def _prepare_in_maps(inputs):
    f = np.float32
    memory = np.ascontiguousarray(inputs["memory"], dtype=f)
    ctrl = np.ascontiguousarray(inputs["controller_state"], dtype=f)
    wc0 = np.ascontiguousarray(
        np.concatenate([inputs["Wk_r"], inputs["Wk_w"]], axis=1), dtype=f)
    wc1 = np.ascontiguousarray(
        np.concatenate([inputs["We"], inputs["Wa"]], axis=1), dtype=f)
    wc2 = np.ascontiguousarray(np.concatenate(
        [inputs["Wb_r"], inputs["Wg_r"], inputs["Wgam_r"],
         inputs["Wb_w"], inputs["Wg_w"], inputs["Wgam_w"],
         inputs["Ws_r"], inputs["Ws_w"]], axis=1), dtype=f)
    b2 = np.concatenate(
        [inputs["bb_r"], inputs["bg_r"], inputs["bgam_r"],
         inputs["bb_w"], inputs["bg_w"], inputs["bgam_w"],
         inputs["bs_r"], inputs["bs_w"]]).astype(f).reshape(12, 1)
    beba = np.concatenate([inputs["be"], inputs["ba"]]).astype(f).reshape(128, 1)
    iv = np.array([[inputs["init_r"][0, 0], inputs["init_w"][0, 0]]], dtype=f)

    in_maps = []
    for i in range(NCORES):
        in_maps.append({
            "mem": np.ascontiguousarray(memory[i * BL:(i + 1) * BL]),
            "ctrlT": np.ascontiguousarray(ctrl[i * BL:(i + 1) * BL].T),
            "wc0": wc0, "wc1": wc1, "wc2": wc2,
            "b2": b2, "beba": beba, "iv": iv,
        })
    return in_maps


def kernel(**inputs):
    nc = _get_program()
    in_maps = _prepare_in_maps(inputs)
    res = run_bass_kernel_spmd(nc, in_maps, core_ids=list(range(NCORES)))
    newmem = np.concatenate([r["newmem"] for r in res.results], axis=0)
    readc = np.concatenate([r["readc"] for r in res.results], axis=0)
    if DEBUG:
        kernel.debug = res.results
    return readc, newmem
